# revision 1
# baseline (speedup 1.0000x reference)
"""DSVF kernel for trn2: biquad SVF applied via FFT overlap-add in the
reference == exact causal 64-tap FIR (poles |z|=0.426 -> h decays below
fp32 eps by tap ~32).  Implemented as Toeplitz matmuls on TensorE.

Layout per core (8 rows of 262144):
  - nat[p, :]  = x[row, p*2048:(p+1)*2048]   (contiguous DMA, 8KB/partition)
  - 16 PE transposes per row -> X~[q, j*128+p] = chunk(16p+j)[q]
  - psY[m, s] = sum_q A[q,m] X~[q,s] + sum_q B[q,m] X~[q, s_prev(s)]
      A[q,m] = h[m-q],  B[q,m] = h[m-q+128]
      s_prev = s-128 for j>=1; for j==0 (s=p): chunk 16p-1 lives at
      storage 1920+p-1 -> one extra "seam" matmul on cols [1:128);
      col 0 of each row has no previous chunk (zero) -> skipped.
  - 16 PE transposes back -> natural layout -> contiguous DMA out.

Raw bass (not Tile): per-engine programs with standalone wait_ge's —
PE matmul ISA structs only fit ONE attached sync wait, which Tile's
auto-assigned multi-waits violate.

Engine plan per row r:
  SP  : in-DMA nat[r%2]                        (waits transposes r-2 done)
  PE  : 16 transposes -> px[g%2] (4 groups), then out-transposes of row
        r-1 (4 groups, bank order 1,2,3,0), then 4 matmul banks
        (order 1,2,3,0 so the k=0 seam's g3 dependency comes last)
  DVE : 4 copies px->xt[r%2], then 4 copies po->nato[(r-1)%2]
  ACT : out-DMA row r-1, then 4 copies py->ysb[r%2]
"""

import os
import numpy as np

BATCH = 64
L = 262144
N_CORES = 8
ROWS = BATCH // N_CORES  # 8 rows per core
P = 128
M = L // P  # 2048 columns per row in natural SBUF layout
NBLK = M // P  # 16 transpose blocks per row
K_TAPS = 64
# matmul dtype mode (measured on HW, full kernel):
#   "f32"   3.1e-7 rel err (fp32 noise floor), ~110.5us
#   "bf16s" hi/lo-split bf16, 8.9e-6 rel err,  ~109.5us
#   "f32r"  ~2.6e-4 rel err (tf32-like rounding), not worth it
# f32 matches the reference's own rounding noise; the 1us bf16s gain is
# not worth any threshold risk.
MM_MODE = os.environ.get("DSVF_MM_MODE", "f32")
TRACE = os.environ.get("DSVF_TRACE", "0") == "1"

_cache = {}

# MM bank order: k=0 carries the seam matmul that needs transpose group 3,
# so it goes last; banks 1..3 only need groups <= their own index.
KSEQ = [1, 2, 3, 0]


def _taps(g_param, R_param, m_hp, m_bp, m_lp):
    """64-tap impulse response of the biquad, float64 host math."""
    g = np.tan(np.pi * (1.0 / (1.0 + np.exp(-np.float64(g_param)))) / 2.0)
    R = np.log1p(np.exp(np.float64(R_param)))
    g2 = g * g
    b = [g2 * m_lp + g * m_bp + m_hp,
         2 * g2 * m_lp - 2 * m_hp,
         g2 * m_lp - g * m_bp + m_hp]
    a = [g2 + 2 * R * g + 1, 2 * g2 - 2, g2 - 2 * R * g + 1]
    h = np.zeros(K_TAPS, np.float64)
    for n in range(K_TAPS):
        acc = 0.0
        if n < 3:
            acc += b[n]
        if n >= 1:
            acc -= a[1] * h[n - 1]
        if n >= 2:
            acc -= a[2] * h[n - 2]
        h[n] = acc / a[0]
    return h


def _toeplitz_mats(h):
    A = np.zeros((P, P), np.float32)  # A[q, m] = h[m-q]
    B = np.zeros((P, P), np.float32)  # B[q, m] = h[m-q+128]
    for q in range(P):
        for m in range(P):
            d = m - q
            if 0 <= d < K_TAPS:
                A[q, m] = h[d]
            d2 = m - q + P
            if 0 < d2 < K_TAPS:
                B[q, m] = h[d2]
    return A, B


def _build(mm_mode):
    import concourse.bass as bass
    import concourse.mybir as mybir
    from contextlib import ExitStack

    f32 = mybir.dt.float32
    f32r = mybir.dt.float32r
    bf16 = mybir.dt.bfloat16

    nc = bass.Bass()
    x = nc.declare_dram_parameter("x", [ROWS, L], f32, isOutput=False)
    tid = nc.declare_dram_parameter("tid", [P, P], f32, isOutput=False)
    if mm_mode == "bf16s":
        tah = nc.declare_dram_parameter("tah", [P, P], bf16, isOutput=False)
        tal = nc.declare_dram_parameter("tal", [P, P], bf16, isOutput=False)
        tbh = nc.declare_dram_parameter("tbh", [P, P], bf16, isOutput=False)
        tbl = nc.declare_dram_parameter("tbl", [P, P], bf16, isOutput=False)
        n_cst = 80
    else:
        ta = nc.declare_dram_parameter("ta", [P, P], f32, isOutput=False)
        tb = nc.declare_dram_parameter("tb", [P, P], f32, isOutput=False)
        n_cst = 48
    y = nc.declare_dram_parameter("y", [ROWS, L], f32, isOutput=True)

    xv = x.rearrange("r (p m) -> r p m", p=P)
    yv = y.rearrange("r (p m) -> r p m", p=P)

    def mmc(ap):
        return ap.bitcast(f32r) if mm_mode == "f32r" else ap

    with ExitStack() as st:
        ident = st.enter_context(nc.sbuf_tensor("ident", [P, P], f32))
        if mm_mode == "bf16s":
            ah_sb = st.enter_context(nc.sbuf_tensor("ah_sb", [P, P], bf16))
            al_sb = st.enter_context(nc.sbuf_tensor("al_sb", [P, P], bf16))
            bh_sb = st.enter_context(nc.sbuf_tensor("bh_sb", [P, P], bf16))
            bl_sb = st.enter_context(nc.sbuf_tensor("bl_sb", [P, P], bf16))
            xh = [st.enter_context(nc.sbuf_tensor(f"xh{i}", [P, M + 1], bf16))
                  for i in range(2)]
            xl = [st.enter_context(nc.sbuf_tensor(f"xl{i}", [P, M + 1], bf16))
                  for i in range(2)]
        else:
            a_sb = st.enter_context(nc.sbuf_tensor("a_sb", [P, P], f32))
            b_sb = st.enter_context(nc.sbuf_tensor("b_sb", [P, P], f32))
        if mm_mode == "f32r":
            # f32r matmul operands must be produced pre-rounded to f32r
            a_r = st.enter_context(nc.sbuf_tensor("a_r", [P, P], f32r))
            b_r = st.enter_context(nc.sbuf_tensor("b_r", [P, P], f32r))
        nat = [st.enter_context(nc.sbuf_tensor(f"nat{i}", [P, M], f32))
               for i in range(2)]
        # xt layout: storage col s<1920 at s; a permanent ZERO guard col at
        # 1920 (so the j==0 seam matmul is an aligned N=128: out col p reads
        # col 1920+p = chunk 16p-1, with p=0 hitting the zero); block 15
        # (s>=1920) shifted to cols 1921..2048.
        xt = [st.enter_context(nc.sbuf_tensor(f"xt{i}", [P, M + 1], f32))
              for i in range(2)]
        ysb = [st.enter_context(nc.sbuf_tensor(f"ysb{i}", [P, M], f32))
               for i in range(2)]
        nato = [st.enter_context(nc.sbuf_tensor(f"nato{i}", [P, M], f32))
                for i in range(2)]
        px = [st.enter_context(nc.psum_tensor(f"px{i}", [P, 512], f32))
              for i in range(3)]
        py = [st.enter_context(nc.psum_tensor(f"py{i}", [P, 512], f32))
              for i in range(2)]
        po = [st.enter_context(nc.psum_tensor(f"po{i}", [P, 512], f32))
              for i in range(3)]

        sInit = st.enter_context(nc.semaphore("sInit"))
        dCst = st.enter_context(nc.semaphore("dCst"))
        # DMA-completion sems: a dma_start's 16 increments come from 16
        # independent SDMA engines, so two in-flight transfers sharing one
        # sem can interleave increments.  Row-0 chunks each get their own
        # sem; later rows alternate by parity (same-parity transfers are
        # serialized by the nat WAR wait, so no mixing).
        dC = [st.enter_context(nc.semaphore(f"dC{g}")) for g in range(4)]
        dInP = [st.enter_context(nc.semaphore(f"dInP{i}")) for i in range(2)]
        dOutP = [st.enter_context(nc.semaphore(f"dOutP{i}")) for i in range(2)]
        sTp = st.enter_context(nc.semaphore("sTp"))    # +1 per in-transpose group
        sXt = st.enter_context(nc.semaphore("sXt"))    # +1 per px->xt copy
        sMm = st.enter_context(nc.semaphore("sMm"))    # +1 per finished MM bank
        sYc = st.enter_context(nc.semaphore("sYc"))    # +1 per py->ysb copy
        sOt = st.enter_context(nc.semaphore("sOt"))    # +1 per out-transpose group
        sNc = st.enter_context(nc.semaphore("sNc"))    # +1 per po->nato copy

        blk = st.enter_context(nc.Block())

        @blk.sync
        def _(sp):
            # row 0 in 4 chunks so PE can start transposing at ~1.5us;
            # consts go down the ACT HWDGE ring in parallel
            for g in range(4):
                sp.dma_start(out=nat[0][:, g * 512:(g + 1) * 512],
                             in_=xv[0][:, g * 512:(g + 1) * 512]
                             ).then_inc(dC[g], 16)
            for r in range(1, ROWS):
                if r >= 2:
                    # nat[r%2] still being read by row r-2 transposes
                    sp.wait_ge(sTp, 4 * (r - 2) + 4)
                sp.dma_start(out=nat[r % 2][:],
                             in_=xv[r]).then_inc(dInP[r % 2], 16)

        if mm_mode == "bf16s":
            a_terms = [(ah_sb, "h"), (ah_sb, "l"), (al_sb, "h")]
            b_terms = [(bh_sb, "h"), (bh_sb, "l"), (bl_sb, "h")]
        else:
            lhs_a = a_r if mm_mode == "f32r" else a_sb
            lhs_b = b_r if mm_mode == "f32r" else b_sb
            a_terms = [(None, "x")]
            b_terms = [(None, "x")]

        def emit_mm(pe, r, out_ap, terms, is_a, lo, hi, first, last):
            """Emit the term-set of one logical matmul on moving cols
            [lo:hi) of the chunk storage; first => opens the PSUM group."""
            n = len(terms)
            for t, (S, which) in enumerate(terms):
                if mm_mode == "bf16s":
                    mov = (xh if which == "h" else xl)[r % 2][:, lo:hi]
                else:
                    S = lhs_a if is_a else lhs_b
                    mov = mmc(xt[r % 2][:, lo:hi])
                ins = pe.matmul(out_ap, S[:], mov,
                                start=(first and t == 0),
                                stop=(last and t == n - 1))
            return ins

        def pe_out_transpose(pe, r1, i, k):
            """out-transpose group i of row r1, reading ysb bank k."""
            oo = 4 * r1 + i
            pe.wait_ge(sYc, 4 * r1 + i + 1)
            if oo >= 3:
                pe.wait_ge(sNc, oo - 2)  # po[oo%3] freed by nato copy oo-3
            dst = po[oo % 3]
            for jj in range(4):
                ins = pe.transpose(
                    dst[:, jj * P:(jj + 1) * P],
                    ysb[r1 % 2][:, (4 * k + jj) * P:(4 * k + jj + 1) * P],
                    ident[:])
            ins.then_inc(sOt, 1)

        @blk.tensor
        def _(pe):
            pe.wait_ge(dCst, n_cst)  # ident + filter matrices
            if mm_mode == "f32r":
                pe.wait_ge(sInit, 1)
            for r in range(ROWS):
                # in-transposes: group g covers blocks 4g..4g+3
                for g in range(4):
                    gg = 4 * r + g
                    if r == 0:
                        pe.wait_ge(dC[g], 16)
                    elif g == 0:
                        # rows of this parity seen so far (row 0 uses dC)
                        pcnt = (r + 1) // 2 if r % 2 else r // 2
                        pe.wait_ge(dInP[r % 2], 16 * pcnt)
                    if gg >= 3:
                        pe.wait_ge(sXt, gg - 2)  # px[gg%3] freed by copy gg-3
                    dst = px[gg % 3]
                    for jj in range(4):
                        j = 4 * g + jj
                        ins = pe.transpose(
                            dst[:, jj * P:(jj + 1) * P],
                            nat[r % 2][:, j * P:(j + 1) * P],
                            ident[:])
                    ins.then_inc(sTp, 1)  # tick 4r+g+1
                # out-transposes of row r-1 first (their inputs are long
                # ready), then a DENSE matmul phase: transpose-mode gets no
                # HAM activity credit, so interleaving them with the MMs
                # would keep the PE clock throttled at 1.2 GHz.
                if r >= 1:
                    for i, k in enumerate(KSEQ):
                        pe_out_transpose(pe, r - 1, i, k)
                for i, k in enumerate(KSEQ):
                    bb = 4 * r + i
                    need_g = 4 if k == 0 else k + 1
                    pe.wait_ge(sXt, 4 * r + need_g)
                    if bb >= 2:
                        pe.wait_ge(sYc, bb - 1)  # py[i%2] freed
                    out = py[i % 2]
                    c0 = k * 512
                    if k == 3:
                        # A-range spans the zero-guard insert at col 1920.
                        # Only the very first matmul starts the group
                        # (start=True clears the whole bank); later regions'
                        # elements are unwritten, so start=False overwrites.
                        emit_mm(pe, r, out[:, 0:384], a_terms, True,
                                1536, 1920, True, False)
                        emit_mm(pe, r, out[:, 384:512], a_terms, True,
                                1921, 2049, False, False)
                    else:
                        emit_mm(pe, r, out[:], a_terms, True,
                                c0, c0 + 512, True, False)
                    if k == 0:
                        emit_mm(pe, r, out[:, 128:512], b_terms, False,
                                0, 384, False, False)
                        # seam: out col p reads col 1920+p (chunk 16p-1;
                        # p=0 reads the permanent zero column)
                        ins = emit_mm(pe, r, out[:, 0:128], b_terms, False,
                                      1920, 2048, False, True)
                    else:
                        ins = emit_mm(pe, r, out[:], b_terms, False,
                                      c0 - 128, c0 + 384, False, True)
                    ins.then_inc(sMm, 1)
            # final row out-transposes
            for i, k in enumerate(KSEQ):
                pe_out_transpose(pe, ROWS - 1, i, k)

        @blk.vector
        def _(dve):
            if mm_mode == "f32r":
                dve.wait_ge(dCst, 48)
                dve.tensor_copy(a_r[:], a_sb[:])
                dve.tensor_copy(b_r[:], b_sb[:]).then_inc(sInit, 1)
            # permanent seam guard columns
            if mm_mode == "bf16s":
                for t in (xh[0], xh[1], xl[0], xl[1]):
                    dve.memset(t[:, 1920:1921], 0.0)
            else:
                dve.memset(xt[0][:, 1920:1921], 0.0)
                dve.memset(xt[1][:, 1920:1921], 0.0)

            def stage_x(r, pxs, lo, hi, plo, phi, inc):
                """Move px[plo:phi) into chunk storage cols [lo:hi)."""
                if mm_mode == "bf16s":
                    dve.tensor_copy(xh[r % 2][:, lo:hi], pxs[:, plo:phi])
                    ins = dve.tensor_sub(xl[r % 2][:, lo:hi], pxs[:, plo:phi],
                                         xh[r % 2][:, lo:hi])
                else:
                    ins = dve.tensor_copy(mmc(xt[r % 2][:, lo:hi]),
                                          pxs[:, plo:phi])
                if inc:
                    ins.then_inc(sXt, 1)

            for r in range(ROWS):
                if r >= 2:
                    dve.wait_ge(sMm, 4 * (r - 2) + 4)  # x bufs still read
                for g in range(4):
                    gg = 4 * r + g
                    dve.wait_ge(sTp, gg + 1)
                    if g == 3:
                        # block 15 lands after the zero-guard column
                        stage_x(r, px[gg % 3][:], 1536, 1920, 0, 384, False)
                        stage_x(r, px[gg % 3][:], 1921, 2049, 384, 512, True)
                    else:
                        stage_x(r, px[gg % 3][:], g * 512, (g + 1) * 512,
                                0, 512, True)

        @blk.scalar
        def _(act):
            act.dma_start(out=ident[:], in_=tid[:]).then_inc(dCst, 16)
            if mm_mode == "bf16s":
                act.dma_start(out=ah_sb[:], in_=tah[:]).then_inc(dCst, 16)
                act.dma_start(out=al_sb[:], in_=tal[:]).then_inc(dCst, 16)
                act.dma_start(out=bh_sb[:], in_=tbh[:]).then_inc(dCst, 16)
                act.dma_start(out=bl_sb[:], in_=tbl[:]).then_inc(dCst, 16)
            else:
                act.dma_start(out=a_sb[:], in_=ta[:]).then_inc(dCst, 16)
                act.dma_start(out=b_sb[:], in_=tb[:]).then_inc(dCst, 16)
            for r in range(ROWS):
                # nato copies of row r-1 first — the PE emits row r-1's
                # out-transposes before row r's matmuls, and outT(r-1,3)
                # waits on nato-copy(r-1,0) for its PSUM bank
                if r >= 1:
                    for i, k in enumerate(KSEQ):
                        oo = 4 * (r - 1) + i
                        if i == 0 and r >= 3:
                            # nato[(r-1)%2] still being DMA'd out (row r-3)
                            j = r - 3
                            ocnt = j // 2 + 1 if j % 2 == 0 else (j + 1) // 2
                            act.wait_ge(dOutP[j % 2], 16 * ocnt)
                        act.wait_ge(sOt, oo + 1)
                        act.copy(out=nato[(r - 1) % 2][:, k * 512:(k + 1) * 512],
                                 in_=po[oo % 3][:]).then_inc(sNc, 1)
                    act.wait_ge(sNc, 4 * (r - 1) + 4)
                    act.dma_start(out=yv[r - 1], in_=nato[(r - 1) % 2][:]
                                  ).then_inc(dOutP[(r - 1) % 2], 16)
                if r >= 2:
                    act.wait_ge(sOt, 4 * (r - 2) + 4)  # ysb[r%2] still read
                for i, k in enumerate(KSEQ):
                    act.wait_ge(sMm, 4 * r + i + 1)
                    act.copy(out=ysb[r % 2][:, k * 512:(k + 1) * 512],
                             in_=py[i % 2][:]).then_inc(sYc, 1)
            # final row: copy + store per 512-block to shorten the tail
            r1 = ROWS - 1
            j = ROWS - 3  # previous occupant of nato[r1%2]
            act.wait_ge(dOutP[j % 2], 16 * (j // 2 + 1 if j % 2 == 0
                                            else (j + 1) // 2))
            for i, k in enumerate(KSEQ):
                oo = 4 * r1 + i
                act.wait_ge(sOt, oo + 1)
                act.copy(out=nato[r1 % 2][:, k * 512:(k + 1) * 512],
                         in_=po[oo % 3][:]).then_inc(sNc, 1)
                act.dma_start(out=yv[r1][:, k * 512:(k + 1) * 512],
                              in_=nato[r1 % 2][:, k * 512:(k + 1) * 512]
                              ).then_inc(dOutP[r1 % 2], 16)
            # rows 0,2,4,6 on parity 0 = 64; rows 1,3,5 + 4 chunks = 112
            act.wait_ge(dOutP[0], 64)
            act.wait_ge(dOutP[1], 112)

    return nc


def _get_nc():
    key = MM_MODE
    if key not in _cache:
        _cache[key] = _build(MM_MODE)
    return _cache[key]


def kernel(**inputs):
    from concourse.bass_utils import run_bass_kernel_spmd

    x = np.ascontiguousarray(np.asarray(inputs["x"], dtype=np.float32))
    assert x.shape == (BATCH, L), x.shape
    h = _taps(float(np.asarray(inputs["g_param"]).reshape(-1)[0]),
              float(np.asarray(inputs["R_param"]).reshape(-1)[0]),
              float(np.asarray(inputs["m_hp"]).reshape(-1)[0]),
              float(np.asarray(inputs["m_bp"]).reshape(-1)[0]),
              float(np.asarray(inputs["m_lp"]).reshape(-1)[0]))
    A, B = _toeplitz_mats(h)
    ident = np.eye(P, dtype=np.float32)
    common = {"tid": ident}
    if MM_MODE == "bf16s":
        import ml_dtypes
        bf = ml_dtypes.bfloat16
        common["tah"] = A.astype(bf)
        common["tal"] = (A - common["tah"].astype(np.float32)).astype(bf)
        common["tbh"] = B.astype(bf)
        common["tbl"] = (B - common["tbh"].astype(np.float32)).astype(bf)
    else:
        common["ta"] = A
        common["tb"] = B

    nc = _get_nc()
    core_ids = list(range(N_CORES))
    in_maps = [
        {"x": x[i * ROWS:(i + 1) * ROWS], **common}
        for i in range(N_CORES)
    ]
    kwargs = {}
    if TRACE:
        kwargs["tmpdir"] = os.environ.get("DSVF_TRACE_DIR") or None
    res = run_bass_kernel_spmd(nc, in_maps, core_ids, trace=TRACE, **kwargs)
    if TRACE:
        kernel.last_exec_time_ns = res.exec_time_ns
        kernel.last_results = res
    out = np.concatenate([res.results[i]["y"] for i in range(N_CORES)], axis=0)
    return out.astype(np.float32, copy=False)


kernel.last_exec_time_ns = None



# revision 13
# speedup vs baseline: 1.2144x; 1.2144x over previous
"""DSVF kernel for trn2: biquad SVF via FFT overlap-add in the reference
== exact causal 64-tap FIR (poles |z|=0.426 -> h decays below fp32 eps
by tap ~32).  Implemented as Toeplitz matmuls on TensorE.

v2 layout (vs v1's interleaved-output + out-transposes): the matmul
uses the TRANSPOSED DATA as the stationary operand and the Toeplitz
pair as the moving operand, so the output lands directly in natural
layout -- the 16 out-transposes per row and their PSUM->SBUF copies
disappear.  All PE ops run in f32r (single-pass tf32-like, ~1.5e-4
matmul rel err, measured 2.6e-4 end-to-end on v1; gate is 2e-2).

Math: per batch row, x natural nat[p, 2048] (p = 128 partitions of
2048).  Chunk c = 16p + j covers x[2048p + 128j .. +128).  In-transpose
of natural 128-block j gives xt[q, 128j + p] = chunk(16p+j)[q].
For output natural block b (cols [128b, 128b+128) of every partition):
  stationary xt[:, 128b:128b+128]  (chunk(16m+b) in column m)
  A-move:  out[m, s] += sum_q h[s-q]     x[2048m + 128b + q]   (block b)
  B-move:  out[m, s] += sum_q h[s+128-q] x[2048m + 128b + q]   (block b+1)
Block b's PSUM region accumulates B (from stationary b-1, start=1)
then A (from stationary b, start=0): no vector adds needed.
Seam: block 0's B-contribution comes from chunks 16m-1 (end of the
previous partition row) = block-15 columns shifted by one partition,
realized by a stationary window over [1920:2048) with a permanent
zero-guard column at 1920 (block 15 is stored shifted to 1921..2049).
Block 0 accumulates A(0) at row start and seam-B at row end in a
dedicated PSUM region.

Raw bass (not Tile): PE matmul ISA structs only fit ONE attached sync
wait, which Tile's auto-assigned multi-waits violate.

Engine plan per row r:
  SP  : in-DMA nat[r%2]                   (waits transposes r-2 done)
  PE  : 16 transposes -> px[g%3] (4 groups), then 32 matmuls
        (pairs per stationary j: A(j), B(j+1); j=15: A(15), seam)
  DVE : 4 copies px->xt[r%2]
  ACT : 5 copies pyr/pyz->nato (batches {1-4}{5-8}{9-12}{13-15}{0}),
        out-DMA row
"""

import os
import numpy as np

BATCH = 64
L = 262144
N_CORES = 8
ROWS = BATCH // N_CORES  # 8 rows per core
P = 128
M = L // P  # 2048 columns per row in natural SBUF layout
NBLK = M // P  # 16 blocks per row
K_TAPS = 64
# "f32r": single-pass tf32-like matmuls (fast path, ~2.6e-4 rel err)
# "f32" : two-pass fp32 (fallback, ~3e-7 rel err)
MM_MODE = os.environ.get("DSVF_MM_MODE", "f32r")
TRACE = os.environ.get("DSVF_TRACE", "0") == "1"

_cache = {}


def _taps(g_param, R_param, m_hp, m_bp, m_lp):
    """64-tap impulse response of the biquad, float64 host math."""
    g = np.tan(np.pi * (1.0 / (1.0 + np.exp(-np.float64(g_param)))) / 2.0)
    R = np.log1p(np.exp(np.float64(R_param)))
    g2 = g * g
    b = [g2 * m_lp + g * m_bp + m_hp,
         2 * g2 * m_lp - 2 * m_hp,
         g2 * m_lp - g * m_bp + m_hp]
    a = [g2 + 2 * R * g + 1, 2 * g2 - 2, g2 - 2 * R * g + 1]
    h = np.zeros(K_TAPS, np.float64)
    for n in range(K_TAPS):
        acc = 0.0
        if n < 3:
            acc += b[n]
        if n >= 1:
            acc -= a[1] * h[n - 1]
        if n >= 2:
            acc -= a[2] * h[n - 2]
        h[n] = acc / a[0]
    return h


def _toeplitz_mats(h):
    A = np.zeros((P, P), np.float32)  # A[q, s] = h[s-q]
    B = np.zeros((P, P), np.float32)  # B[q, s] = h[s-q+128]
    for q in range(P):
        for m in range(P):
            d = m - q
            if 0 <= d < K_TAPS:
                A[q, m] = h[d]
            d2 = m - q + P
            if 0 < d2 < K_TAPS:
                B[q, m] = h[d2]
    return A, B


def _build(mm_mode):
    import concourse.bass as bass
    import concourse.mybir as mybir
    from contextlib import ExitStack

    f32 = mybir.dt.float32
    f32r = mybir.dt.float32r
    rmode = mm_mode == "f32r"
    # transposes also run single-pass f32r (via bitcast views of the f32
    # nat/ident: the verifier only rejects f32r-DECLARED tensors with
    # non-rounding producers); the px->xt DVE copy then does the true
    # f32 -> f32r rounding that the matmul stationary requires.
    dt_px = f32r if rmode else f32
    dt_xt = f32r if rmode else f32

    nc = bass.Bass()
    x = nc.declare_dram_parameter("x", [ROWS, L], f32, isOutput=False)
    tid = nc.declare_dram_parameter("tid", [P, P], f32, isOutput=False)
    tab = nc.declare_dram_parameter("tab", [P, 2 * P], f32, isOutput=False)
    y = nc.declare_dram_parameter("y", [ROWS, L], f32, isOutput=True)

    xv = x.rearrange("r (p m) -> r p m", p=P)
    yv = y.rearrange("r (p m) -> r p m", p=P)

    def xb(ap):
        # f32-bit view of f32r storage
        return ap.bitcast(f32) if rmode else ap

    def rb(ap):
        # f32r view of f32 storage
        return ap.bitcast(f32r) if rmode else ap

    with ExitStack() as st:
        ident_f = st.enter_context(nc.sbuf_tensor("ident_f", [P, P], f32))
        tab_f = st.enter_context(nc.sbuf_tensor("tab_f", [P, 2 * P], f32))
        if rmode:
            tabr = st.enter_context(nc.sbuf_tensor("tab_r", [P, 2 * P], f32r))
            identr = st.enter_context(nc.sbuf_tensor("ident_rr", [P, P], f32r))
        else:
            tabr, identr = tab_f, ident_f
        nat = [st.enter_context(nc.sbuf_tensor(f"nat{i}", [P, M], dt_px))
               for i in range(2)]
        # xt: blocks 0..14 at [128j, 128j+128); permanent zero-guard col
        # at 1920; block 15 shifted to 1921..2049.
        xt = [st.enter_context(nc.sbuf_tensor(f"xt{i}", [P, M + 1], dt_xt))
              for i in range(2)]
        nato = [st.enter_context(nc.sbuf_tensor(f"nato{i}", [P, M], f32))
                for i in range(2)]
        px = [st.enter_context(nc.psum_tensor(f"px{i}", [P, 512], dt_px))
              for i in range(3)]
        # pyr: 8-slot ring (128 cols each) for blocks 1..15, slot (b-1)%8
        pyr = [st.enter_context(nc.psum_tensor(f"pyr{i}", [P, 512], f32))
               for i in range(2)]
        # pyz: block 0 region (A at row start + seam-B at row end)
        pyz = st.enter_context(nc.psum_tensor("pyz", [P, P], f32))

        dCst = st.enter_context(nc.semaphore("dCst"))
        sInit = st.enter_context(nc.semaphore("sInit"))
        # DMA-completion sems: row-0 chunks each get their own sem; later
        # rows alternate by parity (same-parity transfers serialized by
        # the nat WAR wait, so no increment mixing).
        dC = [st.enter_context(nc.semaphore(f"dC{g}")) for g in range(4)]
        dInP = [st.enter_context(nc.semaphore(f"dInP{i}")) for i in range(2)]
        dOutP = [st.enter_context(nc.semaphore(f"dOutP{i}")) for i in range(2)]
        sTp = st.enter_context(nc.semaphore("sTp"))  # +1 per transpose group
        sXt = st.enter_context(nc.semaphore("sXt"))  # +1 per px->xt copy group
        sMm = st.enter_context(nc.semaphore("sMm"))  # +1 per matmul
        sCp = st.enter_context(nc.semaphore("sCp"))  # +1 per ACT copy batch

        blk = st.enter_context(nc.Block())

        @blk.sync
        def _(sp):
            # row 0 in 4 chunks so PE can start transposing early;
            # consts go down the ACT HWDGE ring in parallel
            for g in range(4):
                sp.dma_start(out=nat[0][:, g * 512:(g + 1) * 512],
                             in_=rb(xv[0][:, g * 512:(g + 1) * 512])
                             ).then_inc(dC[g], 16)
            for r in range(1, ROWS):
                if r >= 2:
                    sp.wait_ge(sTp, 4 * (r - 2) + 4)  # nat[r%2] free
                sp.dma_start(out=nat[r % 2][:],
                             in_=rb(xv[r])).then_inc(dInP[r % 2], 16)

        def slot(b):
            s = (b - 1) % 8
            return pyr[s // 4][:, (s % 4) * P:(s % 4 + 1) * P]

        @blk.tensor
        def _(pe):
            pe.wait_ge(dCst, 32)
            pe.wait_ge(sInit, 1)
            for r in range(ROWS):
                # in-transposes: group g covers blocks 4g..4g+3
                for g in range(4):
                    gg = 4 * r + g
                    if r == 0:
                        pe.wait_ge(dC[g], 16)
                    elif g == 0:
                        pcnt = (r + 1) // 2 if r % 2 else r // 2
                        pe.wait_ge(dInP[r % 2], 16 * pcnt)
                    if gg >= 3:
                        pe.wait_ge(sXt, gg - 2)  # px[gg%3] freed
                    dst = px[gg % 3]
                    for jj in range(4):
                        j = 4 * g + jj
                        ins = pe.transpose(
                            dst[:, jj * P:(jj + 1) * P],
                            nat[r % 2][:, j * P:(j + 1) * P],
                            identr[:])
                    ins.then_inc(sTp, 1)
                # matmul pairs per stationary j: A(j) then B(j+1)/seam.
                # mm idx within row: A(b)=2b, B(b)=2b-1, seam=31.
                for j in range(NBLK):
                    pe.wait_ge(sXt, 4 * r + j // 4 + 1)
                    if j == 0:
                        # pyz free: all row r-1 copies done
                        if r >= 1:
                            pe.wait_ge(sCp, 5 * r)
                        lhs = xt[r % 2][:, 0:P]
                        ins = pe.matmul(pyz[:], lhs, tabr[:, 0:P],
                                        start=True, stop=False,
                                        skip_group_check=True)
                        ins.then_inc(sMm, 1)  # A(0)
                        ins = pe.matmul(slot(1), lhs, tabr[:, P:2 * P],
                                        start=True, stop=False,
                                        skip_group_check=True)
                        ins.then_inc(sMm, 1)  # B(1)
                    elif j < NBLK - 1:
                        # slot-reuse gates (ring depth 8):
                        #   j==8: B(9)->slot0, prev block 1 (copy batch 1)
                        #   j==12..14: slots 4..6, prev blocks 5..7 (batch 2)
                        if j == 8:
                            pe.wait_ge(sCp, 5 * r + 1)
                        elif j == 12:
                            pe.wait_ge(sCp, 5 * r + 2)
                        lhs = xt[r % 2][:, j * P:(j + 1) * P]
                        ins = pe.matmul(slot(j), lhs, tabr[:, 0:P],
                                        start=False, stop=True,
                                        skip_group_check=True)
                        ins.then_inc(sMm, 1)  # A(j)
                        ins = pe.matmul(slot(j + 1), lhs, tabr[:, P:2 * P],
                                        start=True, stop=False,
                                        skip_group_check=True)
                        ins.then_inc(sMm, 1)  # B(j+1)
                    else:
                        # j==15: A(15) from shifted block 15, then seam-B
                        ins = pe.matmul(slot(15), xt[r % 2][:, 1921:2049],
                                        tabr[:, 0:P],
                                        start=False, stop=True,
                                        skip_group_check=True)
                        ins.then_inc(sMm, 1)  # A(15)
                        ins = pe.matmul(pyz[:], xt[r % 2][:, 1920:2048],
                                        tabr[:, P:2 * P],
                                        start=False, stop=True,
                                        skip_group_check=True)
                        ins.then_inc(sMm, 1)  # seam -> block 0

        @blk.vector
        def _(dve):
            dve.wait_ge(dCst, 32)
            if rmode:
                dve.tensor_copy(tabr[:], tab_f[:])
                dve.tensor_copy(identr[:], ident_f[:])
            # permanent seam guard columns
            dve.memset(xb(xt[0][:, 1920:1921]), 0.0)
            ins = dve.memset(xb(xt[1][:, 1920:1921]), 0.0)
            ins.then_inc(sInit, 1)

            for r in range(ROWS):
                if r >= 2:
                    dve.wait_ge(sMm, 32 * (r - 1))  # xt[r%2] still read
                for g in range(4):
                    gg = 4 * r + g
                    dve.wait_ge(sTp, gg + 1)
                    # px holds f32 bits; out dtype f32r => this copy IS
                    # the f32->f32r rounding for the matmul stationary
                    pxs = xb(px[gg % 3][:])
                    if g == 3:
                        dve.tensor_copy(xt[r % 2][:, 1536:1920],
                                        pxs[:, 0:384])
                        ins = dve.tensor_copy(xt[r % 2][:, 1921:2049],
                                              pxs[:, 384:512])
                    else:
                        ins = dve.tensor_copy(
                            xt[r % 2][:, g * 512:(g + 1) * 512], pxs)
                    ins.then_inc(sXt, 1)

        @blk.scalar
        def _(act):
            act.dma_start(out=ident_f[:], in_=tid[:]).then_inc(dCst, 16)
            act.dma_start(out=tab_f[:], in_=tab[:]).then_inc(dCst, 16)
            for r in range(ROWS):
                last = r == ROWS - 1
                # nato[r%2] free: out-DMA of row r-2 done (2 DMAs x 16;
                # last row uses 5)
                if r >= 2:
                    act.wait_ge(dOutP[r % 2], 32 * (r // 2))
                # copy batches: {1-4}{5-8}{9-12}{13-15}{0}
                batches = [
                    (32 * r + 9, pyr[0][:, 0:512], 1, 4),
                    (32 * r + 17, pyr[1][:, 0:512], 5, 4),
                    (32 * r + 25, pyr[0][:, 0:512], 9, 4),
                    (32 * r + 31, pyr[1][:, 0:384], 13, 3),
                    (32 * r + 32, pyz[:], 0, 1),
                ]
                for (mmw, src, b0, nb) in batches:
                    act.wait_ge(sMm, mmw)
                    ins = act.copy(out=nato[r % 2][:, b0 * P:(b0 + nb) * P],
                                   in_=src)
                    ins.then_inc(sCp, 1)
                    if last and b0 != 0:
                        act.dma_start(out=yv[r][:, b0 * P:(b0 + nb) * P],
                                      in_=nato[r % 2][:, b0 * P:(b0 + nb) * P]
                                      ).then_inc(dOutP[r % 2], 16)
                    elif not last and b0 == 13:
                        # cols [128:2048) complete after batch 4
                        act.dma_start(out=yv[r][:, P:M],
                                      in_=nato[r % 2][:, P:M]
                                      ).then_inc(dOutP[r % 2], 16)
                act.dma_start(out=yv[r][:, 0:P],
                              in_=nato[r % 2][:, 0:P]
                              ).then_inc(dOutP[r % 2], 16)
            # drain: parity 0 rows 0,2,4,6 = 4*32; parity 1 rows 1,3,5 =
            # 3*32 plus last row 7's 5 DMAs = 80
            act.wait_ge(dOutP[0], 128)
            act.wait_ge(dOutP[1], 176)

    return nc


def _get_nc():
    key = MM_MODE
    if key not in _cache:
        _cache[key] = _build(MM_MODE)
    return _cache[key]


def kernel(**inputs):
    from concourse.bass_utils import run_bass_kernel_spmd

    x = np.ascontiguousarray(np.asarray(inputs["x"], dtype=np.float32))
    assert x.shape == (BATCH, L), x.shape
    h = _taps(float(np.asarray(inputs["g_param"]).reshape(-1)[0]),
              float(np.asarray(inputs["R_param"]).reshape(-1)[0]),
              float(np.asarray(inputs["m_hp"]).reshape(-1)[0]),
              float(np.asarray(inputs["m_bp"]).reshape(-1)[0]),
              float(np.asarray(inputs["m_lp"]).reshape(-1)[0]))
    A, B = _toeplitz_mats(h)
    ident = np.eye(P, dtype=np.float32)
    common = {"tid": ident, "tab": np.concatenate([A, B], axis=1)}

    nc = _get_nc()
    core_ids = list(range(N_CORES))
    in_maps = [
        {"x": x[i * ROWS:(i + 1) * ROWS], **common}
        for i in range(N_CORES)
    ]
    kwargs = {}
    if TRACE:
        kwargs["tmpdir"] = os.environ.get("DSVF_TRACE_DIR") or None
    res = run_bass_kernel_spmd(nc, in_maps, core_ids, trace=TRACE, **kwargs)
    if TRACE:
        kernel.last_exec_time_ns = res.exec_time_ns
        kernel.last_results = res
    out = np.concatenate([res.results[i]["y"] for i in range(N_CORES)], axis=0)
    return out.astype(np.float32, copy=False)


kernel.last_exec_time_ns = None


# revision 19
# speedup vs baseline: 1.5553x; 1.2807x over previous
"""DSVF kernel for trn2: biquad SVF via FFT overlap-add in the reference
== exact causal 64-tap FIR (poles |z|=0.426 -> h decays below fp32 eps
by tap ~32).  Implemented as Toeplitz matmuls on TensorE.

v2 layout (vs v1's interleaved-output + out-transposes): the matmul
uses the TRANSPOSED DATA as the stationary operand and the Toeplitz
pair as the moving operand, so the output lands directly in natural
layout -- the 16 out-transposes per row and their PSUM->SBUF copies
disappear.  All PE ops run in f32r (single-pass tf32-like, ~1.5e-4
matmul rel err, measured 2.6e-4 end-to-end on v1; gate is 2e-2).

Math: per batch row, x natural nat[p, 2048] (p = 128 partitions of
2048).  Chunk c = 16p + j covers x[2048p + 128j .. +128).  In-transpose
of natural 128-block j gives xt[q, 128j + p] = chunk(16p+j)[q].
For output natural block b (cols [128b, 128b+128) of every partition):
  stationary xt[:, 128b:128b+128]  (chunk(16m+b) in column m)
  A-move:  out[m, s] += sum_q h[s-q]     x[2048m + 128b + q]   (block b)
  B-move:  out[m, s] += sum_q h[s+128-q] x[2048m + 128b + q]   (block b+1)
Block b's PSUM region accumulates B (from stationary b-1, start=1)
then A (from stationary b, start=0): no vector adds needed.
Seam: block 0's B-contribution comes from chunks 16m-1 (end of the
previous partition row) = block-15 columns shifted by one partition,
realized by a stationary window over [1920:2048) with a permanent
zero-guard column at 1920 (block 15 is stored shifted to 1921..2049).
Block 0 accumulates A(0) at row start and seam-B at row end in a
dedicated PSUM region.

Raw bass (not Tile): PE matmul ISA structs only fit ONE attached sync
wait, which Tile's auto-assigned multi-waits violate.

Engine plan per row r:
  SP  : in-DMA nat[r%2]                   (waits transposes r-2 done)
  PE  : 16 transposes -> px[g%3] (4 groups), then 32 matmuls
        (pairs per stationary j: A(j), B(j+1); j=15: A(15), seam)
  DVE : 4 copies px->xt[r%2]
  ACT : 5 copies pyr/pyz->nato (batches {1-4}{5-8}{9-12}{13-15}{0}),
        out-DMA row
"""

import os
import numpy as np

BATCH = 64
L = 262144
N_CORES = 8
ROWS = BATCH // N_CORES  # 8 rows per core
P = 128
M = L // P  # 2048 columns per row in natural SBUF layout
NBLK = M // P  # 16 blocks per row
K_TAPS = 64
# "bf16": bf16 Toeplitz matmuls (fast path, ~3.8e-3 rel err, gate 2e-2)
# "f32r": single-pass tf32-like matmuls (~2.6e-4 rel err)
# "f32" : two-pass fp32 (~3e-7 rel err)
MM_MODE = os.environ.get("DSVF_MM_MODE", "bf16")
TRACE = os.environ.get("DSVF_TRACE", "0") == "1"

_cache = {}


def _taps(g_param, R_param, m_hp, m_bp, m_lp):
    """64-tap impulse response of the biquad, float64 host math."""
    g = np.tan(np.pi * (1.0 / (1.0 + np.exp(-np.float64(g_param)))) / 2.0)
    R = np.log1p(np.exp(np.float64(R_param)))
    g2 = g * g
    b = [g2 * m_lp + g * m_bp + m_hp,
         2 * g2 * m_lp - 2 * m_hp,
         g2 * m_lp - g * m_bp + m_hp]
    a = [g2 + 2 * R * g + 1, 2 * g2 - 2, g2 - 2 * R * g + 1]
    h = np.zeros(K_TAPS, np.float64)
    for n in range(K_TAPS):
        acc = 0.0
        if n < 3:
            acc += b[n]
        if n >= 1:
            acc -= a[1] * h[n - 1]
        if n >= 2:
            acc -= a[2] * h[n - 2]
        h[n] = acc / a[0]
    return h


def _toeplitz_mats(h):
    A = np.zeros((P, P), np.float32)  # A[q, s] = h[s-q]
    B = np.zeros((P, P), np.float32)  # B[q, s] = h[s-q+128]
    for q in range(P):
        for m in range(P):
            d = m - q
            if 0 <= d < K_TAPS:
                A[q, m] = h[d]
            d2 = m - q + P
            if 0 < d2 < K_TAPS:
                B[q, m] = h[d2]
    return A, B


def _build(mm_mode):
    import concourse.bass as bass
    import concourse.mybir as mybir
    from contextlib import ExitStack

    f32 = mybir.dt.float32
    f32r = mybir.dt.float32r
    bf16 = mybir.dt.bfloat16
    rmode = mm_mode == "f32r"
    bmode = mm_mode == "bf16"
    # transposes run single-pass f32r (nat declared f32r, DMA'd via f32r
    # APs so the producer dtype satisfies the verifier); the px->xt DVE
    # copy does the true f32 -> f32r/bf16 rounding for the matmul
    # stationary.
    dt_px = f32r if (rmode or bmode) else f32
    dt_xt = bf16 if bmode else (f32r if rmode else f32)

    nc = bass.Bass()
    x = nc.declare_dram_parameter("x", [ROWS, L], f32, isOutput=False)
    tid = nc.declare_dram_parameter("tid", [P, P], f32, isOutput=False)
    tab = nc.declare_dram_parameter("tab", [P, 2 * P], f32, isOutput=False)
    y = nc.declare_dram_parameter("y", [ROWS, L], f32, isOutput=True)

    xv = x.rearrange("r (p m) -> r p m", p=P)
    yv = y.rearrange("r (p m) -> r p m", p=P)

    def xb(ap):
        # f32-bit view of f32r storage
        return ap.bitcast(f32) if (rmode or bmode) else ap

    def rb(ap):
        # f32r view of f32 storage
        return ap.bitcast(f32r) if (rmode or bmode) else ap

    with ExitStack() as st:
        ident_f = st.enter_context(nc.sbuf_tensor("ident_f", [P, P], f32))
        tab_f = st.enter_context(nc.sbuf_tensor("tab_f", [P, 2 * P], f32))
        if bmode:
            tabr = st.enter_context(nc.sbuf_tensor("tab_b", [P, 2 * P], bf16))
            identr = st.enter_context(nc.sbuf_tensor("ident_rr", [P, P], f32r))
        elif rmode:
            tabr = st.enter_context(nc.sbuf_tensor("tab_r", [P, 2 * P], f32r))
            identr = st.enter_context(nc.sbuf_tensor("ident_rr", [P, P], f32r))
        else:
            tabr, identr = tab_f, ident_f
        nat = [st.enter_context(nc.sbuf_tensor(f"nat{i}", [P, M], dt_px))
               for i in range(2)]
        # xt: blocks 0..14 at [128j, 128j+128); permanent zero-guard col
        # at 1920; block 15 shifted to 1921..2049.
        xt = [st.enter_context(nc.sbuf_tensor(f"xt{i}", [P, M + 1], dt_xt))
              for i in range(2)]
        nato = [st.enter_context(nc.sbuf_tensor(f"nato{i}", [P, M], f32))
                for i in range(2)]
        px = [st.enter_context(nc.psum_tensor(f"px{i}", [P, 512], dt_px))
              for i in range(3)]
        # pyr: 8-slot ring (128 cols each) for blocks 1..15, slot (b-1)%8
        pyr = [st.enter_context(nc.psum_tensor(f"pyr{i}", [P, 512], f32))
               for i in range(2)]
        # pyz: block 0 region (A at row start + seam-B at row end)
        pyz = st.enter_context(nc.psum_tensor("pyz", [P, P], f32))

        dCst = st.enter_context(nc.semaphore("dCst"))
        sInit = st.enter_context(nc.semaphore("sInit"))
        # DMA-completion sems: row-0 chunks each get their own sem; later
        # rows alternate by parity (same-parity transfers serialized by
        # the nat WAR wait, so no increment mixing).
        dC = [st.enter_context(nc.semaphore(f"dC{g}")) for g in range(4)]
        dInP = [st.enter_context(nc.semaphore(f"dInP{i}")) for i in range(2)]
        dOutP = [st.enter_context(nc.semaphore(f"dOutP{i}")) for i in range(2)]
        sTp = st.enter_context(nc.semaphore("sTp"))  # +1 per transpose group
        sXt = st.enter_context(nc.semaphore("sXt"))  # +1 per px->xt copy group
        sMm = st.enter_context(nc.semaphore("sMm"))  # +1 per matmul
        sCp = st.enter_context(nc.semaphore("sCp"))  # +1 per ACT copy batch

        blk = st.enter_context(nc.Block())

        @blk.sync
        def _(sp):
            # row 0 in 4 chunks so PE can start transposing early;
            # consts go down the ACT HWDGE ring in parallel
            for g in range(4):
                sp.dma_start(out=nat[0][:, g * 512:(g + 1) * 512],
                             in_=rb(xv[0][:, g * 512:(g + 1) * 512])
                             ).then_inc(dC[g], 16)
            for r in range(1, ROWS):
                if r >= 2:
                    sp.wait_ge(sTp, 4 * (r - 2) + 4)  # nat[r%2] free
                sp.dma_start(out=nat[r % 2][:],
                             in_=rb(xv[r])).then_inc(dInP[r % 2], 16)

        def slot(b):
            s = (b - 1) % 8
            return pyr[s // 4][:, (s % 4) * P:(s % 4 + 1) * P]

        @blk.tensor
        def _(pe):
            pe.wait_ge(dCst, 32)
            pe.wait_ge(sInit, 1)
            for r in range(ROWS):
                # in-transposes: group g covers blocks 4g..4g+3
                for g in range(4):
                    gg = 4 * r + g
                    if r == 0:
                        pe.wait_ge(dC[g], 16)
                    elif g == 0:
                        pcnt = (r + 1) // 2 if r % 2 else r // 2
                        pe.wait_ge(dInP[r % 2], 16 * pcnt)
                    if gg >= 3:
                        pe.wait_ge(sXt, gg - 2)  # px[gg%3] freed
                    dst = px[gg % 3]
                    for jj in range(4):
                        j = 4 * g + jj
                        ins = pe.transpose(
                            dst[:, jj * P:(jj + 1) * P],
                            nat[r % 2][:, j * P:(j + 1) * P],
                            identr[:])
                    ins.then_inc(sTp, 1)
                # matmul pairs per stationary j: A(j) then B(j+1)/seam.
                # mm idx within row: A(b)=2b, B(b)=2b-1, seam=31.
                for j in range(NBLK):
                    pe.wait_ge(sXt, 4 * r + j // 4 + 1)
                    if j == 0:
                        # pyz free: all row r-1 copies done
                        if r >= 1:
                            pe.wait_ge(sCp, 5 * r)
                        lhs = xt[r % 2][:, 0:P]
                        ins = pe.matmul(pyz[:], lhs, tabr[:, 0:P],
                                        start=True, stop=False,
                                        skip_group_check=True)
                        ins.then_inc(sMm, 1)  # A(0)
                        ins = pe.matmul(slot(1), lhs, tabr[:, P:2 * P],
                                        start=True, stop=False,
                                        skip_group_check=True)
                        ins.then_inc(sMm, 1)  # B(1)
                    elif j < NBLK - 1:
                        # slot-reuse gates (ring depth 8):
                        #   j==8: B(9)->slot0, prev block 1 (copy batch 1)
                        #   j==12..14: slots 4..6, prev blocks 5..7 (batch 2)
                        if j == 8:
                            pe.wait_ge(sCp, 5 * r + 1)
                        elif j == 12:
                            pe.wait_ge(sCp, 5 * r + 2)
                        lhs = xt[r % 2][:, j * P:(j + 1) * P]
                        ins = pe.matmul(slot(j), lhs, tabr[:, 0:P],
                                        start=False, stop=True,
                                        skip_group_check=True)
                        ins.then_inc(sMm, 1)  # A(j)
                        ins = pe.matmul(slot(j + 1), lhs, tabr[:, P:2 * P],
                                        start=True, stop=False,
                                        skip_group_check=True)
                        ins.then_inc(sMm, 1)  # B(j+1)
                    else:
                        # j==15: A(15) from shifted block 15, then seam-B
                        ins = pe.matmul(slot(15), xt[r % 2][:, 1921:2049],
                                        tabr[:, 0:P],
                                        start=False, stop=True,
                                        skip_group_check=True)
                        ins.then_inc(sMm, 1)  # A(15)
                        ins = pe.matmul(pyz[:], xt[r % 2][:, 1920:2048],
                                        tabr[:, P:2 * P],
                                        start=False, stop=True,
                                        skip_group_check=True)
                        ins.then_inc(sMm, 1)  # seam -> block 0

        @blk.vector
        def _(dve):
            dve.wait_ge(dCst, 32)
            if rmode or bmode:
                dve.tensor_copy(tabr[:], tab_f[:])
                dve.tensor_copy(identr[:], ident_f[:])
            # permanent seam guard columns
            gv = (lambda ap: ap.bitcast(f32)) if rmode else (lambda ap: ap)
            dve.memset(gv(xt[0][:, 1920:1921]), 0.0)
            ins = dve.memset(gv(xt[1][:, 1920:1921]), 0.0)
            ins.then_inc(sInit, 1)

            for r in range(ROWS):
                if r >= 2:
                    dve.wait_ge(sMm, 32 * (r - 1))  # xt[r%2] still read
                for g in range(4):
                    gg = 4 * r + g
                    dve.wait_ge(sTp, gg + 1)
                    # px holds f32 bits; out dtype f32r => this copy IS
                    # the f32->f32r rounding for the matmul stationary
                    pxs = xb(px[gg % 3][:])
                    if g == 3:
                        dve.tensor_copy(xt[r % 2][:, 1536:1920],
                                        pxs[:, 0:384])
                        ins = dve.tensor_copy(xt[r % 2][:, 1921:2049],
                                              pxs[:, 384:512])
                    else:
                        ins = dve.tensor_copy(
                            xt[r % 2][:, g * 512:(g + 1) * 512], pxs)
                    ins.then_inc(sXt, 1)

        @blk.scalar
        def _(act):
            act.dma_start(out=ident_f[:], in_=tid[:]).then_inc(dCst, 16)
            act.dma_start(out=tab_f[:], in_=tab[:]).then_inc(dCst, 16)
            for r in range(ROWS):
                last = r == ROWS - 1
                # nato[r%2] free: out-DMA of row r-2 done (2 DMAs x 16;
                # last row uses 5)
                if r >= 2:
                    act.wait_ge(dOutP[r % 2], 32 * (r // 2))
                # copy batches: {1-4}{5-8}{9-12}{13-15}{0}
                batches = [
                    (32 * r + 9, pyr[0][:, 0:512], 1, 4),
                    (32 * r + 17, pyr[1][:, 0:512], 5, 4),
                    (32 * r + 25, pyr[0][:, 0:512], 9, 4),
                    (32 * r + 31, pyr[1][:, 0:384], 13, 3),
                    (32 * r + 32, pyz[:], 0, 1),
                ]
                for (mmw, src, b0, nb) in batches:
                    act.wait_ge(sMm, mmw)
                    ins = act.copy(out=nato[r % 2][:, b0 * P:(b0 + nb) * P],
                                   in_=src)
                    ins.then_inc(sCp, 1)
                    if last and b0 != 0:
                        act.dma_start(out=yv[r][:, b0 * P:(b0 + nb) * P],
                                      in_=nato[r % 2][:, b0 * P:(b0 + nb) * P]
                                      ).then_inc(dOutP[r % 2], 16)
                    elif not last and b0 == 13:
                        # cols [128:2048) complete after batch 4
                        act.dma_start(out=yv[r][:, P:M],
                                      in_=nato[r % 2][:, P:M]
                                      ).then_inc(dOutP[r % 2], 16)
                act.dma_start(out=yv[r][:, 0:P],
                              in_=nato[r % 2][:, 0:P]
                              ).then_inc(dOutP[r % 2], 16)
            # drain: parity 0 rows 0,2,4,6 = 4*32; parity 1 rows 1,3,5 =
            # 3*32 plus last row 7's 5 DMAs = 80
            act.wait_ge(dOutP[0], 128)
            act.wait_ge(dOutP[1], 176)

    return nc


def _get_nc():
    key = MM_MODE
    if key not in _cache:
        _cache[key] = _build(MM_MODE)
    return _cache[key]


def kernel(**inputs):
    from concourse.bass_utils import run_bass_kernel_spmd

    x = np.ascontiguousarray(np.asarray(inputs["x"], dtype=np.float32))
    assert x.shape == (BATCH, L), x.shape
    h = _taps(float(np.asarray(inputs["g_param"]).reshape(-1)[0]),
              float(np.asarray(inputs["R_param"]).reshape(-1)[0]),
              float(np.asarray(inputs["m_hp"]).reshape(-1)[0]),
              float(np.asarray(inputs["m_bp"]).reshape(-1)[0]),
              float(np.asarray(inputs["m_lp"]).reshape(-1)[0]))
    A, B = _toeplitz_mats(h)
    ident = np.eye(P, dtype=np.float32)
    common = {"tid": ident, "tab": np.concatenate([A, B], axis=1)}

    nc = _get_nc()
    core_ids = list(range(N_CORES))
    in_maps = [
        {"x": x[i * ROWS:(i + 1) * ROWS], **common}
        for i in range(N_CORES)
    ]
    kwargs = {}
    if TRACE:
        kwargs["tmpdir"] = os.environ.get("DSVF_TRACE_DIR") or None
    res = run_bass_kernel_spmd(nc, in_maps, core_ids, trace=TRACE, **kwargs)
    if TRACE:
        kernel.last_exec_time_ns = res.exec_time_ns
        kernel.last_results = res
    out = np.concatenate([res.results[i]["y"] for i in range(N_CORES)], axis=0)
    return out.astype(np.float32, copy=False)


kernel.last_exec_time_ns = None


# revision 23
# speedup vs baseline: 1.7567x; 1.1294x over previous
"""DSVF kernel for trn2: biquad SVF via FFT overlap-add in the reference
== exact causal 64-tap FIR (poles |z|=0.426 -> h decays below fp32 eps
by tap ~32).  Implemented as Toeplitz matmuls on TensorE.

v2 layout (vs v1's interleaved-output + out-transposes): the matmul
uses the TRANSPOSED DATA as the stationary operand and the Toeplitz
pair as the moving operand, so the output lands directly in natural
layout -- the 16 out-transposes per row and their PSUM->SBUF copies
disappear.  All PE ops run in f32r (single-pass tf32-like, ~1.5e-4
matmul rel err, measured 2.6e-4 end-to-end on v1; gate is 2e-2).

Math: per batch row, x natural nat[p, 2048] (p = 128 partitions of
2048).  Chunk c = 16p + j covers x[2048p + 128j .. +128).  In-transpose
of natural 128-block j gives xt[q, 128j + p] = chunk(16p+j)[q].
For output natural block b (cols [128b, 128b+128) of every partition):
  stationary xt[:, 128b:128b+128]  (chunk(16m+b) in column m)
  A-move:  out[m, s] += sum_q h[s-q]     x[2048m + 128b + q]   (block b)
  B-move:  out[m, s] += sum_q h[s+128-q] x[2048m + 128b + q]   (block b+1)
Block b's PSUM region accumulates B (from stationary b-1, start=1)
then A (from stationary b, start=0): no vector adds needed.
Seam: block 0's B-contribution comes from chunks 16m-1 (end of the
previous partition row) = block-15 columns shifted by one partition,
realized by a stationary window over [1920:2048) with a permanent
zero-guard column at 1920 (block 15 is stored shifted to 1921..2049).
Block 0 accumulates A(0) at row start and seam-B at row end in a
dedicated PSUM region.

Raw bass (not Tile): PE matmul ISA structs only fit ONE attached sync
wait, which Tile's auto-assigned multi-waits violate.

Engine plan per row r:
  SP  : in-DMA nat[r%2]                   (waits transposes r-2 done)
  PE  : 16 transposes -> px[g%3] (4 groups), then 32 matmuls
        (pairs per stationary j: A(j), B(j+1); j=15: A(15), seam)
  DVE : 4 copies px->xt[r%2]
  ACT : 5 copies pyr/pyz->nato (batches {1-4}{5-8}{9-12}{13-15}{0}),
        out-DMA row
"""

import os
import numpy as np

BATCH = 64
L = 262144
N_CORES = 8
ROWS = BATCH // N_CORES  # 8 rows per core
P = 128
M = L // P  # 2048 columns per row in natural SBUF layout
NBLK = M // P  # 16 blocks per row
K_TAPS = 64
# "bf16": bf16 Toeplitz matmuls (fast path, ~3.8e-3 rel err, gate 2e-2)
# "f32r": single-pass tf32-like matmuls (~2.6e-4 rel err)
# "f32" : two-pass fp32 (~3e-7 rel err)
MM_MODE = os.environ.get("DSVF_MM_MODE", "bf16")
TRACE = os.environ.get("DSVF_TRACE", "0") == "1"

_cache = {}


def _taps(g_param, R_param, m_hp, m_bp, m_lp):
    """64-tap impulse response of the biquad, float64 host math."""
    g = np.tan(np.pi * (1.0 / (1.0 + np.exp(-np.float64(g_param)))) / 2.0)
    R = np.log1p(np.exp(np.float64(R_param)))
    g2 = g * g
    b = [g2 * m_lp + g * m_bp + m_hp,
         2 * g2 * m_lp - 2 * m_hp,
         g2 * m_lp - g * m_bp + m_hp]
    a = [g2 + 2 * R * g + 1, 2 * g2 - 2, g2 - 2 * R * g + 1]
    h = np.zeros(K_TAPS, np.float64)
    for n in range(K_TAPS):
        acc = 0.0
        if n < 3:
            acc += b[n]
        if n >= 1:
            acc -= a[1] * h[n - 1]
        if n >= 2:
            acc -= a[2] * h[n - 2]
        h[n] = acc / a[0]
    return h


def _toeplitz_mats(h):
    A = np.zeros((P, P), np.float32)  # A[q, s] = h[s-q]
    B = np.zeros((P, P), np.float32)  # B[q, s] = h[s-q+128]
    for q in range(P):
        for m in range(P):
            d = m - q
            if 0 <= d < K_TAPS:
                A[q, m] = h[d]
            d2 = m - q + P
            if 0 < d2 < K_TAPS:
                B[q, m] = h[d2]
    return A, B


def _build(mm_mode):
    import concourse.bass as bass
    import concourse.mybir as mybir
    from contextlib import ExitStack

    f32 = mybir.dt.float32
    f32r = mybir.dt.float32r
    bf16 = mybir.dt.bfloat16
    rmode = mm_mode == "f32r"
    bmode = mm_mode == "bf16"
    # transposes run single-pass f32r (nat declared f32r, DMA'd via f32r
    # APs so the producer dtype satisfies the verifier); the px->xt DVE
    # copy does the true f32 -> f32r/bf16 rounding for the matmul
    # stationary.
    dt_px = f32r if (rmode or bmode) else f32
    dt_xt = bf16 if bmode else (f32r if rmode else f32)

    nc = bass.Bass()
    x = nc.declare_dram_parameter("x", [ROWS, L], f32, isOutput=False)
    tid = nc.declare_dram_parameter("tid", [P, P], f32, isOutput=False)
    tab = nc.declare_dram_parameter("tab", [P, 2 * P], f32, isOutput=False)
    y = nc.declare_dram_parameter("y", [ROWS, L], f32, isOutput=True)

    xv = x.rearrange("r (p m) -> r p m", p=P)
    yv = y.rearrange("r (p m) -> r p m", p=P)

    def xb(ap):
        # f32-bit view of f32r storage
        return ap.bitcast(f32) if (rmode or bmode) else ap

    def rb(ap):
        # f32r view of f32 storage
        return ap.bitcast(f32r) if (rmode or bmode) else ap

    with ExitStack() as st:
        ident_f = st.enter_context(nc.sbuf_tensor("ident_f", [P, P], f32))
        tab_f = st.enter_context(nc.sbuf_tensor("tab_f", [P, 2 * P], f32))
        if bmode:
            tabr = st.enter_context(nc.sbuf_tensor("tab_b", [P, 2 * P], bf16))
            identr = st.enter_context(nc.sbuf_tensor("ident_rr", [P, P], f32r))
        elif rmode:
            tabr = st.enter_context(nc.sbuf_tensor("tab_r", [P, 2 * P], f32r))
            identr = st.enter_context(nc.sbuf_tensor("ident_rr", [P, P], f32r))
        else:
            tabr, identr = tab_f, ident_f
        # one nat buffer per row: input prefetches with no pipeline coupling
        nat = [st.enter_context(nc.sbuf_tensor(f"nat{i}", [P, M], dt_px))
               for i in range(ROWS)]
        # xt: blocks 0..14 at [128j, 128j+128); permanent zero-guard col
        # at 1920; block 15 shifted to 1921..2049.
        xt = [st.enter_context(nc.sbuf_tensor(f"xt{i}", [P, M + 1], dt_xt))
              for i in range(2)]
        nato = [st.enter_context(nc.sbuf_tensor(f"nato{i}", [P, M], f32))
                for i in range(4)]
        px = [st.enter_context(nc.psum_tensor(f"px{i}", [P, 512], dt_px))
              for i in range(3)]
        # pyr: 12-slot ring (128 cols each) for blocks 1..15, slot (b-1)%12
        pyr = [st.enter_context(nc.psum_tensor(f"pyr{i}", [P, 512], f32))
               for i in range(3)]
        # pyz: block 0 region (A at row start + seam-B at row end),
        # double-buffered by row parity
        pyz = st.enter_context(nc.psum_tensor("pyz", [P, 2 * P], f32))

        dCst = st.enter_context(nc.semaphore("dCst"))
        sInit = st.enter_context(nc.semaphore("sInit"))
        # per-transfer DMA-completion sems (16 incs each)
        dC = [st.enter_context(nc.semaphore(f"dC{g}")) for g in range(4)]
        dIn = [st.enter_context(nc.semaphore(f"dIn{r}"))
               for r in range(1, ROWS)]
        dOut = [st.enter_context(nc.semaphore(f"dOut{i}")) for i in range(4)]
        sTp = st.enter_context(nc.semaphore("sTp"))  # +1 per transpose group
        sXt = st.enter_context(nc.semaphore("sXt"))  # +1 per px->xt copy group
        sMm = st.enter_context(nc.semaphore("sMm"))  # +1 per matmul
        sCp = st.enter_context(nc.semaphore("sCp"))  # +1 per ACT copy batch

        blk = st.enter_context(nc.Block())

        @blk.sync
        def _(sp):
            # row 0 in 4 chunks so PE can start transposing early; all
            # other rows stream back-to-back (dedicated buffers, no WAR)
            for g in range(4):
                sp.dma_start(out=nat[0][:, g * 512:(g + 1) * 512],
                             in_=rb(xv[0][:, g * 512:(g + 1) * 512])
                             ).then_inc(dC[g], 16)
            for r in range(1, ROWS):
                sp.dma_start(out=nat[r][:],
                             in_=rb(xv[r])).then_inc(dIn[r - 1], 16)

        def slot(b):
            s = (b - 1) % 12
            return pyr[s // 4][:, (s % 4) * P:(s % 4 + 1) * P]

        def pyzr(r):
            return pyz[:, (r % 2) * P:(r % 2 + 1) * P]

        @blk.tensor
        def _(pe):
            pe.wait_ge(dCst, 32)
            pe.wait_ge(sInit, 1)
            for r in range(ROWS):
                # in-transposes: group g covers blocks 4g..4g+3
                for g in range(4):
                    gg = 4 * r + g
                    if r == 0:
                        pe.wait_ge(dC[g], 16)
                    elif g == 0:
                        pe.wait_ge(dIn[r - 1], 16)
                    if gg >= 3:
                        pe.wait_ge(sXt, gg - 2)  # px[gg%3] freed
                    dst = px[gg % 3]
                    for jj in range(4):
                        j = 4 * g + jj
                        ins = pe.transpose(
                            dst[:, jj * P:(jj + 1) * P],
                            nat[r][:, j * P:(j + 1) * P],
                            identr[:])
                    ins.then_inc(sTp, 1)
                # matmul pairs per stationary j: A(j) then B(j+1)/seam.
                # mm idx within row: A(b)=2b, B(b)=2b-1, seam=31.
                for j in range(NBLK):
                    pe.wait_ge(sXt, 4 * r + j // 4 + 1)
                    if j == 0:
                        # ring slots 0..2 freed by row r-1's copy batch 4
                        # (their last writers were blocks 13..15); pyz
                        # region r%2 freed by row r-2's copy batch 5.
                        if r >= 1:
                            pe.wait_ge(sCp, 5 * r - 1)
                        lhs = xt[r % 2][:, 0:P]
                        ins = pe.matmul(pyzr(r), lhs, tabr[:, 0:P],
                                        start=True, stop=False,
                                        skip_group_check=True)
                        ins.then_inc(sMm, 1)  # A(0)
                        ins = pe.matmul(slot(1), lhs, tabr[:, P:2 * P],
                                        start=True, stop=False,
                                        skip_group_check=True)
                        ins.then_inc(sMm, 1)  # B(1)
                    elif j < NBLK - 1:
                        # j==12: B(13)->slot 0, freed by THIS row's copy
                        # batch 1 (blocks 1..4 -> slots 0..3)
                        if j == 12:
                            pe.wait_ge(sCp, 5 * r + 1)
                        lhs = xt[r % 2][:, j * P:(j + 1) * P]
                        ins = pe.matmul(slot(j), lhs, tabr[:, 0:P],
                                        start=False, stop=True,
                                        skip_group_check=True)
                        ins.then_inc(sMm, 1)  # A(j)
                        ins = pe.matmul(slot(j + 1), lhs, tabr[:, P:2 * P],
                                        start=True, stop=False,
                                        skip_group_check=True)
                        ins.then_inc(sMm, 1)  # B(j+1)
                    else:
                        # j==15: A(15) from shifted block 15, then seam-B
                        ins = pe.matmul(slot(15), xt[r % 2][:, 1921:2049],
                                        tabr[:, 0:P],
                                        start=False, stop=True,
                                        skip_group_check=True)
                        ins.then_inc(sMm, 1)  # A(15)
                        ins = pe.matmul(pyzr(r), xt[r % 2][:, 1920:2048],
                                        tabr[:, P:2 * P],
                                        start=False, stop=True,
                                        skip_group_check=True)
                        ins.then_inc(sMm, 1)  # seam -> block 0

        @blk.vector
        def _(dve):
            dve.wait_ge(dCst, 32)
            if rmode or bmode:
                dve.tensor_copy(tabr[:], tab_f[:])
                dve.tensor_copy(identr[:], ident_f[:])
            # permanent seam guard columns
            gv = (lambda ap: ap.bitcast(f32)) if rmode else (lambda ap: ap)
            dve.memset(gv(xt[0][:, 1920:1921]), 0.0)
            ins = dve.memset(gv(xt[1][:, 1920:1921]), 0.0)
            ins.then_inc(sInit, 1)

            for r in range(ROWS):
                if r >= 2:
                    dve.wait_ge(sMm, 32 * (r - 1))  # xt[r%2] still read
                for g in range(4):
                    gg = 4 * r + g
                    dve.wait_ge(sTp, gg + 1)
                    # px holds f32 bits; out dtype f32r => this copy IS
                    # the f32->f32r rounding for the matmul stationary
                    pxs = xb(px[gg % 3][:])
                    if g == 3:
                        dve.tensor_copy(xt[r % 2][:, 1536:1920],
                                        pxs[:, 0:384])
                        ins = dve.tensor_copy(xt[r % 2][:, 1921:2049],
                                              pxs[:, 384:512])
                    else:
                        ins = dve.tensor_copy(
                            xt[r % 2][:, g * 512:(g + 1) * 512], pxs)
                    ins.then_inc(sXt, 1)

        @blk.scalar
        def _(act):
            act.dma_start(out=ident_f[:], in_=tid[:]).then_inc(dCst, 16)
            act.dma_start(out=tab_f[:], in_=tab[:]).then_inc(dCst, 16)
            for r in range(ROWS):
                # nato[r%4] free: out-DMA of row r-4 done
                if r >= 4:
                    act.wait_ge(dOut[r % 4], 32 * (r // 4))
                # copy batches: {1-4}{5-8}{9-12}{13-15}{0}
                batches = [
                    (32 * r + 9, pyr[0][:, 0:512], 1, 4),
                    (32 * r + 17, pyr[1][:, 0:512], 5, 4),
                    (32 * r + 25, pyr[2][:, 0:512], 9, 4),
                    (32 * r + 31, pyr[0][:, 0:384], 13, 3),
                    (32 * r + 32, pyzr(r), 0, 1),
                ]
                for (mmw, src, b0, nb) in batches:
                    act.wait_ge(sMm, mmw)
                    ins = act.copy(out=nato[r % 4][:, b0 * P:(b0 + nb) * P],
                                   in_=src)
                    ins.then_inc(sCp, 1)

        @blk.gpsimd
        def _(gp):
            # out-DMA triggers on the (otherwise idle) gpsimd HWDGE ring,
            # keeping the scalar engine free for PSUM->SBUF copies
            for r in range(ROWS):
                last = r == ROWS - 1
                if last:
                    # per-batch stores to shorten the tail
                    for k, (b0, nb) in enumerate([(1, 4), (5, 4), (9, 4),
                                                  (13, 3)]):
                        gp.wait_ge(sCp, 5 * r + k + 1)
                        gp.dma_start(out=yv[r][:, b0 * P:(b0 + nb) * P],
                                     in_=nato[r % 4][:, b0 * P:(b0 + nb) * P]
                                     ).then_inc(dOut[r % 4], 16)
                else:
                    gp.wait_ge(sCp, 5 * r + 4)
                    gp.dma_start(out=yv[r][:, P:M],
                                 in_=nato[r % 4][:, P:M]
                                 ).then_inc(dOut[r % 4], 16)
                gp.wait_ge(sCp, 5 * r + 5)
                gp.dma_start(out=yv[r][:, 0:P],
                             in_=nato[r % 4][:, 0:P]
                             ).then_inc(dOut[r % 4], 16)
            # drain: buffers 0..2 served rows {0,4},{1,5},{2,6} = 2x32;
            # buffer 3 rows {3,7} = 32 + 80
            gp.wait_ge(dOut[0], 64)
            gp.wait_ge(dOut[1], 64)
            gp.wait_ge(dOut[2], 64)
            gp.wait_ge(dOut[3], 112)

    return nc


def _get_nc():
    key = MM_MODE
    if key not in _cache:
        _cache[key] = _build(MM_MODE)
    return _cache[key]


def kernel(**inputs):
    from concourse.bass_utils import run_bass_kernel_spmd

    x = np.ascontiguousarray(np.asarray(inputs["x"], dtype=np.float32))
    assert x.shape == (BATCH, L), x.shape
    h = _taps(float(np.asarray(inputs["g_param"]).reshape(-1)[0]),
              float(np.asarray(inputs["R_param"]).reshape(-1)[0]),
              float(np.asarray(inputs["m_hp"]).reshape(-1)[0]),
              float(np.asarray(inputs["m_bp"]).reshape(-1)[0]),
              float(np.asarray(inputs["m_lp"]).reshape(-1)[0]))
    A, B = _toeplitz_mats(h)
    ident = np.eye(P, dtype=np.float32)
    common = {"tid": ident, "tab": np.concatenate([A, B], axis=1)}

    nc = _get_nc()
    core_ids = list(range(N_CORES))
    in_maps = [
        {"x": x[i * ROWS:(i + 1) * ROWS], **common}
        for i in range(N_CORES)
    ]
    kwargs = {}
    if TRACE:
        kwargs["tmpdir"] = os.environ.get("DSVF_TRACE_DIR") or None
    res = run_bass_kernel_spmd(nc, in_maps, core_ids, trace=TRACE, **kwargs)
    if TRACE:
        kernel.last_exec_time_ns = res.exec_time_ns
        kernel.last_results = res
    out = np.concatenate([res.results[i]["y"] for i in range(N_CORES)], axis=0)
    return out.astype(np.float32, copy=False)


kernel.last_exec_time_ns = None


# revision 28
# speedup vs baseline: 1.7712x; 1.0083x over previous
"""DSVF kernel for trn2: biquad SVF via FFT overlap-add in the reference
== exact causal 64-tap FIR (poles |z|=0.426 -> h decays below fp32 eps
by tap ~32).  Implemented as Toeplitz matmuls on TensorE.

v2 layout (vs v1's interleaved-output + out-transposes): the matmul
uses the TRANSPOSED DATA as the stationary operand and the Toeplitz
pair as the moving operand, so the output lands directly in natural
layout -- the 16 out-transposes per row and their PSUM->SBUF copies
disappear.  All PE ops run in f32r (single-pass tf32-like, ~1.5e-4
matmul rel err, measured 2.6e-4 end-to-end on v1; gate is 2e-2).

Math: per batch row, x natural nat[p, 2048] (p = 128 partitions of
2048).  Chunk c = 16p + j covers x[2048p + 128j .. +128).  In-transpose
of natural 128-block j gives xt[q, 128j + p] = chunk(16p+j)[q].
For output natural block b (cols [128b, 128b+128) of every partition):
  stationary xt[:, 128b:128b+128]  (chunk(16m+b) in column m)
  A-move:  out[m, s] += sum_q h[s-q]     x[2048m + 128b + q]   (block b)
  B-move:  out[m, s] += sum_q h[s+128-q] x[2048m + 128b + q]   (block b+1)
Block b's PSUM region accumulates B (from stationary b-1, start=1)
then A (from stationary b, start=0): no vector adds needed.
Seam: block 0's B-contribution comes from chunks 16m-1 (end of the
previous partition row) = block-15 columns shifted by one partition,
realized by a stationary window over [1920:2048) with a permanent
zero-guard column at 1920 (block 15 is stored shifted to 1921..2049).
Block 0 accumulates A(0) at row start and seam-B at row end in a
dedicated PSUM region.

Raw bass (not Tile): PE matmul ISA structs only fit ONE attached sync
wait, which Tile's auto-assigned multi-waits violate.

Engine plan per row r:
  SP  : in-DMA nat[r%2]                   (waits transposes r-2 done)
  PE  : 16 transposes -> px[g%3] (4 groups), then 32 matmuls
        (pairs per stationary j: A(j), B(j+1); j=15: A(15), seam)
  DVE : 4 copies px->xt[r%2]
  ACT : 5 copies pyr/pyz->nato (batches {1-4}{5-8}{9-12}{13-15}{0}),
        out-DMA row
"""

import os
import numpy as np

BATCH = 64
L = 262144
N_CORES = 8
ROWS = BATCH // N_CORES  # 8 rows per core
P = 128
M = L // P  # 2048 columns per row in natural SBUF layout
NBLK = M // P  # 16 blocks per row
K_TAPS = 64
# "bf16": bf16 Toeplitz matmuls (fast path, ~3.8e-3 rel err, gate 2e-2)
# "f32r": single-pass tf32-like matmuls (~2.6e-4 rel err)
# "f32" : two-pass fp32 (~3e-7 rel err)
MM_MODE = os.environ.get("DSVF_MM_MODE", "bf16")
TRACE = os.environ.get("DSVF_TRACE", "0") == "1"

_cache = {}


def _taps(g_param, R_param, m_hp, m_bp, m_lp):
    """64-tap impulse response of the biquad, float64 host math."""
    g = np.tan(np.pi * (1.0 / (1.0 + np.exp(-np.float64(g_param)))) / 2.0)
    R = np.log1p(np.exp(np.float64(R_param)))
    g2 = g * g
    b = [g2 * m_lp + g * m_bp + m_hp,
         2 * g2 * m_lp - 2 * m_hp,
         g2 * m_lp - g * m_bp + m_hp]
    a = [g2 + 2 * R * g + 1, 2 * g2 - 2, g2 - 2 * R * g + 1]
    h = np.zeros(K_TAPS, np.float64)
    for n in range(K_TAPS):
        acc = 0.0
        if n < 3:
            acc += b[n]
        if n >= 1:
            acc -= a[1] * h[n - 1]
        if n >= 2:
            acc -= a[2] * h[n - 2]
        h[n] = acc / a[0]
    return h


def _toeplitz_mats(h):
    A = np.zeros((P, P), np.float32)  # A[q, s] = h[s-q]
    B = np.zeros((P, P), np.float32)  # B[q, s] = h[s-q+128]
    for q in range(P):
        for m in range(P):
            d = m - q
            if 0 <= d < K_TAPS:
                A[q, m] = h[d]
            d2 = m - q + P
            if 0 < d2 < K_TAPS:
                B[q, m] = h[d2]
    return A, B


def _build(mm_mode):
    import concourse.bass as bass
    import concourse.mybir as mybir
    from contextlib import ExitStack

    f32 = mybir.dt.float32
    f32r = mybir.dt.float32r
    bf16 = mybir.dt.bfloat16
    rmode = mm_mode == "f32r"
    bmode = mm_mode == "bf16"
    # bf16 mode: the gpsimd-initiated in-DMA casts f32 -> bf16 inline
    # (only gpsimd DGE can cast), so transposes run entirely in bf16.
    # f32r mode: transposes run single-pass f32r (nat declared f32r,
    # DMA'd via f32r APs to satisfy the verifier); the px->xt DVE copy
    # does the true f32 -> f32r rounding for the matmul stationary.
    dt_px = bf16 if bmode else (f32r if rmode else f32)
    dt_xt = bf16 if bmode else (f32r if rmode else f32)

    nc = bass.Bass()
    x = nc.declare_dram_parameter("x", [ROWS, L], f32, isOutput=False)
    tid = nc.declare_dram_parameter("tid", [P, P], f32, isOutput=False)
    tab = nc.declare_dram_parameter("tab", [P, 2 * P], f32, isOutput=False)
    y = nc.declare_dram_parameter("y", [ROWS, L], f32, isOutput=True)

    xv = x.rearrange("r (p m) -> r p m", p=P)
    yv = y.rearrange("r (p m) -> r p m", p=P)

    def xb(ap):
        # f32-bit view of f32r storage (px reads; bf16 px reads directly)
        return ap.bitcast(f32) if rmode else ap

    def rb(ap):
        # f32r view of f32 dram x (rmode only; bmode uses a casting DMA)
        return ap.bitcast(f32r) if rmode else ap

    with ExitStack() as st:
        ident_f = st.enter_context(nc.sbuf_tensor("ident_f", [P, P], f32))
        tab_f = st.enter_context(nc.sbuf_tensor("tab_f", [P, 2 * P], f32))
        if bmode:
            tabr = st.enter_context(nc.sbuf_tensor("tab_b", [P, 2 * P], bf16))
            identr = st.enter_context(nc.sbuf_tensor("ident_rr", [P, P], bf16))
        elif rmode:
            tabr = st.enter_context(nc.sbuf_tensor("tab_r", [P, 2 * P], f32r))
            identr = st.enter_context(nc.sbuf_tensor("ident_rr", [P, P], f32r))
        else:
            tabr, identr = tab_f, ident_f
        # one nat buffer per row: input prefetches with no pipeline coupling
        nat = [st.enter_context(nc.sbuf_tensor(f"nat{i}", [P, M], dt_px))
               for i in range(ROWS)]
        # xt: blocks 0..14 at [128j, 128j+128); permanent zero-guard col
        # at 1920; block 15 shifted to 1921..2049.
        xt = [st.enter_context(nc.sbuf_tensor(f"xt{i}", [P, M + 1], dt_xt))
              for i in range(2)]
        nato = [st.enter_context(nc.sbuf_tensor(f"nato{i}", [P, M], f32))
                for i in range(4)]
        px = [st.enter_context(nc.psum_tensor(f"px{i}", [P, 512], dt_px))
              for i in range(3)]
        # pyr: 12-slot ring (128 cols each) for blocks 1..15, slot (b-1)%12
        pyr = [st.enter_context(nc.psum_tensor(f"pyr{i}", [P, 512], f32))
               for i in range(3)]
        # pyz: block 0 region (A at row start + seam-B at row end),
        # double-buffered by row parity
        pyz = st.enter_context(nc.psum_tensor("pyz", [P, 2 * P], f32))

        dCst = st.enter_context(nc.semaphore("dCst"))
        sInit = st.enter_context(nc.semaphore("sInit"))
        # per-transfer DMA-completion sems (16 incs each)
        dC = [st.enter_context(nc.semaphore(f"dC{g}")) for g in range(4)]
        dIn = [st.enter_context(nc.semaphore(f"dIn{r}"))
               for r in range(1, ROWS)]
        dOut = [st.enter_context(nc.semaphore(f"dOut{i}")) for i in range(4)]
        sTp = st.enter_context(nc.semaphore("sTp"))  # +1 per transpose group
        sXt = st.enter_context(nc.semaphore("sXt"))  # +1 per px->xt copy group
        sMm = st.enter_context(nc.semaphore("sMm"))  # +1 per matmul
        sCp = st.enter_context(nc.semaphore("sCp"))  # +1 per ACT copy batch

        blk = st.enter_context(nc.Block())

        @blk.gpsimd
        def _(gp):
            # in-DMA on the gpsimd DGE ring: in bf16 mode these casts
            # f32 -> bf16 inline (only gpsimd can initiate casting DMAs).
            # Row 0 in 4 chunks so PE can start transposing early; all
            # other rows stream back-to-back (dedicated buffers, no WAR)
            for g in range(4):
                gp.dma_start(out=nat[0][:, g * 512:(g + 1) * 512],
                             in_=rb(xv[0][:, g * 512:(g + 1) * 512])
                             ).then_inc(dC[g], 16)
            for r in range(1, ROWS):
                gp.dma_start(out=nat[r][:],
                             in_=rb(xv[r])).then_inc(dIn[r - 1], 16)

        def slot(b):
            s = (b - 1) % 12
            return pyr[s // 4][:, (s % 4) * P:(s % 4 + 1) * P]

        def pyzr(r):
            return pyz[:, (r % 2) * P:(r % 2 + 1) * P]

        @blk.tensor
        def _(pe):
            pe.wait_ge(dCst, 32)
            pe.wait_ge(sInit, 1)
            for r in range(ROWS):
                # in-transposes: group g covers blocks 4g..4g+3
                for g in range(4):
                    gg = 4 * r + g
                    if r == 0:
                        pe.wait_ge(dC[g], 16)
                    elif g == 0:
                        pe.wait_ge(dIn[r - 1], 16)
                    if gg >= 3:
                        pe.wait_ge(sXt, gg - 2)  # px[gg%3] freed
                    dst = px[gg % 3]
                    for jj in range(4):
                        j = 4 * g + jj
                        ins = pe.transpose(
                            dst[:, jj * P:(jj + 1) * P],
                            nat[r][:, j * P:(j + 1) * P],
                            identr[:])
                    ins.then_inc(sTp, 1)
                # matmul pairs per stationary j: A(j) then B(j+1)/seam.
                # mm idx within row: A(b)=2b, B(b)=2b-1, seam=31.
                for j in range(NBLK):
                    pe.wait_ge(sXt, 4 * r + j // 4 + 1)
                    if j == 0:
                        # ring slots 0..2 freed by row r-1's copy batch 4
                        # (their last writers were blocks 13..15); pyz
                        # region r%2 freed by row r-2's copy batch 5.
                        if r >= 1:
                            pe.wait_ge(sCp, 5 * r - 1)
                        lhs = xt[r % 2][:, 0:P]
                        ins = pe.matmul(pyzr(r), lhs, tabr[:, 0:P],
                                        start=True, stop=False,
                                        skip_group_check=True)
                        ins.then_inc(sMm, 1)  # A(0)
                        ins = pe.matmul(slot(1), lhs, tabr[:, P:2 * P],
                                        start=True, stop=False,
                                        skip_group_check=True)
                        ins.then_inc(sMm, 1)  # B(1)
                    elif j < NBLK - 1:
                        # j==12: B(13)->slot 0, freed by THIS row's copy
                        # batch 1 (blocks 1..4 -> slots 0..3)
                        if j == 12:
                            pe.wait_ge(sCp, 5 * r + 1)
                        lhs = xt[r % 2][:, j * P:(j + 1) * P]
                        ins = pe.matmul(slot(j), lhs, tabr[:, 0:P],
                                        start=False, stop=True,
                                        skip_group_check=True)
                        ins.then_inc(sMm, 1)  # A(j)
                        ins = pe.matmul(slot(j + 1), lhs, tabr[:, P:2 * P],
                                        start=True, stop=False,
                                        skip_group_check=True)
                        ins.then_inc(sMm, 1)  # B(j+1)
                    else:
                        # j==15: A(15) from shifted block 15, then seam-B
                        ins = pe.matmul(slot(15), xt[r % 2][:, 1921:2049],
                                        tabr[:, 0:P],
                                        start=False, stop=True,
                                        skip_group_check=True)
                        ins.then_inc(sMm, 1)  # A(15)
                        ins = pe.matmul(pyzr(r), xt[r % 2][:, 1920:2048],
                                        tabr[:, P:2 * P],
                                        start=False, stop=True,
                                        skip_group_check=True)
                        ins.then_inc(sMm, 1)  # seam -> block 0

        @blk.vector
        def _(dve):
            dve.wait_ge(dCst, 32)
            if rmode or bmode:
                dve.tensor_copy(tabr[:], tab_f[:])
                dve.tensor_copy(identr[:], ident_f[:])
            # permanent seam guard columns
            gv = (lambda ap: ap.bitcast(f32)) if rmode else (lambda ap: ap)
            dve.memset(gv(xt[0][:, 1920:1921]), 0.0)
            ins = dve.memset(gv(xt[1][:, 1920:1921]), 0.0)
            ins.then_inc(sInit, 1)

            for r in range(ROWS):
                if r >= 2:
                    dve.wait_ge(sMm, 32 * (r - 1))  # xt[r%2] still read
                for g in range(4):
                    gg = 4 * r + g
                    dve.wait_ge(sTp, gg + 1)
                    # px holds f32 bits; out dtype f32r => this copy IS
                    # the f32->f32r rounding for the matmul stationary
                    pxs = xb(px[gg % 3][:])
                    if g == 3:
                        dve.tensor_copy(xt[r % 2][:, 1536:1920],
                                        pxs[:, 0:384])
                        ins = dve.tensor_copy(xt[r % 2][:, 1921:2049],
                                              pxs[:, 384:512])
                    else:
                        ins = dve.tensor_copy(
                            xt[r % 2][:, g * 512:(g + 1) * 512], pxs)
                    ins.then_inc(sXt, 1)

        @blk.scalar
        def _(act):
            act.dma_start(out=ident_f[:], in_=tid[:]).then_inc(dCst, 16)
            act.dma_start(out=tab_f[:], in_=tab[:]).then_inc(dCst, 16)
            for r in range(ROWS):
                # nato[r%4] free: out-DMA of row r-4 done
                if r >= 4:
                    act.wait_ge(dOut[r % 4], 32 * (r // 4))
                # copy batches: {1-4}{5-8}{9-12}{13-15}{0}
                batches = [
                    (32 * r + 9, pyr[0][:, 0:512], 1, 4),
                    (32 * r + 17, pyr[1][:, 0:512], 5, 4),
                    (32 * r + 25, pyr[2][:, 0:512], 9, 4),
                    (32 * r + 31, pyr[0][:, 0:384], 13, 3),
                    (32 * r + 32, pyzr(r), 0, 1),
                ]
                for (mmw, src, b0, nb) in batches:
                    act.wait_ge(sMm, mmw)
                    ins = act.copy(out=nato[r % 4][:, b0 * P:(b0 + nb) * P],
                                   in_=src)
                    ins.then_inc(sCp, 1)

        @blk.sync
        def _(sp):
            # out-DMA triggers on the (otherwise idle) SP HWDGE ring,
            # keeping the scalar engine free for PSUM->SBUF copies
            for r in range(ROWS):
                last = r == ROWS - 1
                if last:
                    # per-batch stores to shorten the tail
                    for k, (b0, nb) in enumerate([(1, 4), (5, 4), (9, 4),
                                                  (13, 3)]):
                        sp.wait_ge(sCp, 5 * r + k + 1)
                        sp.dma_start(out=yv[r][:, b0 * P:(b0 + nb) * P],
                                     in_=nato[r % 4][:, b0 * P:(b0 + nb) * P]
                                     ).then_inc(dOut[r % 4], 16)
                else:
                    sp.wait_ge(sCp, 5 * r + 4)
                    sp.dma_start(out=yv[r][:, P:M],
                                 in_=nato[r % 4][:, P:M]
                                 ).then_inc(dOut[r % 4], 16)
                sp.wait_ge(sCp, 5 * r + 5)
                sp.dma_start(out=yv[r][:, 0:P],
                             in_=nato[r % 4][:, 0:P]
                             ).then_inc(dOut[r % 4], 16)
            # drain: buffers 0..2 served rows {0,4},{1,5},{2,6} = 2x32;
            # buffer 3 rows {3,7} = 32 + 80
            sp.wait_ge(dOut[0], 64)
            sp.wait_ge(dOut[1], 64)
            sp.wait_ge(dOut[2], 64)
            sp.wait_ge(dOut[3], 112)

    return nc


def _get_nc():
    key = MM_MODE
    if key not in _cache:
        _cache[key] = _build(MM_MODE)
    return _cache[key]


def kernel(**inputs):
    from concourse.bass_utils import run_bass_kernel_spmd

    x = np.ascontiguousarray(np.asarray(inputs["x"], dtype=np.float32))
    assert x.shape == (BATCH, L), x.shape
    h = _taps(float(np.asarray(inputs["g_param"]).reshape(-1)[0]),
              float(np.asarray(inputs["R_param"]).reshape(-1)[0]),
              float(np.asarray(inputs["m_hp"]).reshape(-1)[0]),
              float(np.asarray(inputs["m_bp"]).reshape(-1)[0]),
              float(np.asarray(inputs["m_lp"]).reshape(-1)[0]))
    A, B = _toeplitz_mats(h)
    ident = np.eye(P, dtype=np.float32)
    common = {"tid": ident, "tab": np.concatenate([A, B], axis=1)}

    nc = _get_nc()
    core_ids = list(range(N_CORES))
    in_maps = [
        {"x": x[i * ROWS:(i + 1) * ROWS], **common}
        for i in range(N_CORES)
    ]
    kwargs = {}
    if TRACE:
        kwargs["tmpdir"] = os.environ.get("DSVF_TRACE_DIR") or None
    res = run_bass_kernel_spmd(nc, in_maps, core_ids, trace=TRACE, **kwargs)
    if TRACE:
        kernel.last_exec_time_ns = res.exec_time_ns
        kernel.last_results = res
    out = np.concatenate([res.results[i]["y"] for i in range(N_CORES)], axis=0)
    return out.astype(np.float32, copy=False)


kernel.last_exec_time_ns = None


# revision 30
# speedup vs baseline: 1.8107x; 1.0223x over previous
"""DSVF kernel for trn2: biquad SVF via FFT overlap-add in the reference
== exact causal 64-tap FIR (poles |z|=0.426 -> h decays below fp32 eps
by tap ~32).  Implemented as Toeplitz matmuls on TensorE.

v2 layout (vs v1's interleaved-output + out-transposes): the matmul
uses the TRANSPOSED DATA as the stationary operand and the Toeplitz
pair as the moving operand, so the output lands directly in natural
layout -- the 16 out-transposes per row and their PSUM->SBUF copies
disappear.  All PE ops run in f32r (single-pass tf32-like, ~1.5e-4
matmul rel err, measured 2.6e-4 end-to-end on v1; gate is 2e-2).

Math: per batch row, x natural nat[p, 2048] (p = 128 partitions of
2048).  Chunk c = 16p + j covers x[2048p + 128j .. +128).  In-transpose
of natural 128-block j gives xt[q, 128j + p] = chunk(16p+j)[q].
For output natural block b (cols [128b, 128b+128) of every partition):
  stationary xt[:, 128b:128b+128]  (chunk(16m+b) in column m)
  A-move:  out[m, s] += sum_q h[s-q]     x[2048m + 128b + q]   (block b)
  B-move:  out[m, s] += sum_q h[s+128-q] x[2048m + 128b + q]   (block b+1)
Block b's PSUM region accumulates B (from stationary b-1, start=1)
then A (from stationary b, start=0): no vector adds needed.
Seam: block 0's B-contribution comes from chunks 16m-1 (end of the
previous partition row) = block-15 columns shifted by one partition,
realized by a stationary window over [1920:2048) with a permanent
zero-guard column at 1920 (block 15 is stored shifted to 1921..2049).
Block 0 accumulates A(0) at row start and seam-B at row end in a
dedicated PSUM region.

Raw bass (not Tile): PE matmul ISA structs only fit ONE attached sync
wait, which Tile's auto-assigned multi-waits violate.

Engine plan per row r:
  SP  : in-DMA nat[r%2]                   (waits transposes r-2 done)
  PE  : 16 transposes -> px[g%3] (4 groups), then 32 matmuls
        (pairs per stationary j: A(j), B(j+1); j=15: A(15), seam)
  DVE : 4 copies px->xt[r%2]
  ACT : 5 copies pyr/pyz->nato (batches {1-4}{5-8}{9-12}{13-15}{0}),
        out-DMA row
"""

import os
import numpy as np

BATCH = 64
L = 262144
N_CORES = 8
ROWS = BATCH // N_CORES  # 8 rows per core
P = 128
M = L // P  # 2048 columns per row in natural SBUF layout
NBLK = M // P  # 16 blocks per row
K_TAPS = 64
# "bf16": bf16 Toeplitz matmuls (fast path, ~3.8e-3 rel err, gate 2e-2)
# "f32r": single-pass tf32-like matmuls (~2.6e-4 rel err)
# "f32" : two-pass fp32 (~3e-7 rel err)
MM_MODE = os.environ.get("DSVF_MM_MODE", "bf16")
TRACE = os.environ.get("DSVF_TRACE", "0") == "1"

_cache = {}


def _taps(g_param, R_param, m_hp, m_bp, m_lp):
    """64-tap impulse response of the biquad, float64 host math."""
    g = np.tan(np.pi * (1.0 / (1.0 + np.exp(-np.float64(g_param)))) / 2.0)
    R = np.log1p(np.exp(np.float64(R_param)))
    g2 = g * g
    b = [g2 * m_lp + g * m_bp + m_hp,
         2 * g2 * m_lp - 2 * m_hp,
         g2 * m_lp - g * m_bp + m_hp]
    a = [g2 + 2 * R * g + 1, 2 * g2 - 2, g2 - 2 * R * g + 1]
    h = np.zeros(K_TAPS, np.float64)
    for n in range(K_TAPS):
        acc = 0.0
        if n < 3:
            acc += b[n]
        if n >= 1:
            acc -= a[1] * h[n - 1]
        if n >= 2:
            acc -= a[2] * h[n - 2]
        h[n] = acc / a[0]
    return h


def _toeplitz_mats(h):
    A = np.zeros((P, P), np.float32)  # A[q, s] = h[s-q]
    B = np.zeros((P, P), np.float32)  # B[q, s] = h[s-q+128]
    for q in range(P):
        for m in range(P):
            d = m - q
            if 0 <= d < K_TAPS:
                A[q, m] = h[d]
            d2 = m - q + P
            if 0 < d2 < K_TAPS:
                B[q, m] = h[d2]
    return A, B


def _build(mm_mode):
    import concourse.bass as bass
    import concourse.mybir as mybir
    from contextlib import ExitStack

    f32 = mybir.dt.float32
    f32r = mybir.dt.float32r
    bf16 = mybir.dt.bfloat16
    rmode = mm_mode == "f32r"
    bmode = mm_mode == "bf16"
    # bf16 mode: the gpsimd-initiated in-DMA casts f32 -> bf16 inline
    # (only gpsimd DGE can cast), so transposes run entirely in bf16.
    # f32r mode: transposes run single-pass f32r (nat declared f32r,
    # DMA'd via f32r APs to satisfy the verifier); the px->xt DVE copy
    # does the true f32 -> f32r rounding for the matmul stationary.
    dt_px = bf16 if bmode else (f32r if rmode else f32)
    dt_xt = bf16 if bmode else (f32r if rmode else f32)

    nc = bass.Bass()
    x = nc.declare_dram_parameter("x", [ROWS, L], f32, isOutput=False)
    tid = nc.declare_dram_parameter("tid", [P, P], f32, isOutput=False)
    tab = nc.declare_dram_parameter("tab", [P, 2 * P], f32, isOutput=False)
    y = nc.declare_dram_parameter("y", [ROWS, L], f32, isOutput=True)

    xv = x.rearrange("r (p m) -> r p m", p=P)
    yv = y.rearrange("r (p m) -> r p m", p=P)

    def xb(ap):
        # f32-bit view of f32r storage (px reads; bf16 px reads directly)
        return ap.bitcast(f32) if rmode else ap

    def rb(ap):
        # f32r view of f32 dram x (rmode only; bmode uses a casting DMA)
        return ap.bitcast(f32r) if rmode else ap

    with ExitStack() as st:
        ident_f = st.enter_context(nc.sbuf_tensor("ident_f", [P, P], f32))
        tab_f = st.enter_context(nc.sbuf_tensor("tab_f", [P, 2 * P], f32))
        if bmode:
            tabr = st.enter_context(nc.sbuf_tensor("tab_b", [P, 2 * P], bf16))
            identr = st.enter_context(nc.sbuf_tensor("ident_rr", [P, P], bf16))
        elif rmode:
            tabr = st.enter_context(nc.sbuf_tensor("tab_r", [P, 2 * P], f32r))
            identr = st.enter_context(nc.sbuf_tensor("ident_rr", [P, P], f32r))
        else:
            tabr, identr = tab_f, ident_f
        # one nat buffer per row: input prefetches with no pipeline coupling
        nat = [st.enter_context(nc.sbuf_tensor(f"nat{i}", [P, M], dt_px))
               for i in range(ROWS)]
        # xt: blocks 0..14 at [128j, 128j+128); permanent zero-guard col
        # at 1920; block 15 shifted to 1921..2049.
        xt = [st.enter_context(nc.sbuf_tensor(f"xt{i}", [P, M + 1], dt_xt))
              for i in range(2)]
        nato = [st.enter_context(nc.sbuf_tensor(f"nato{i}", [P, M], f32))
                for i in range(4)]
        px = [st.enter_context(nc.psum_tensor(f"px{i}", [P, 512], dt_px))
              for i in range(3)]
        # pyr: 12-slot ring (128 cols each) for blocks 1..15, slot (b-1)%12
        pyr = [st.enter_context(nc.psum_tensor(f"pyr{i}", [P, 512], f32))
               for i in range(3)]
        # pyz: block 0 region (A at row start + seam-B at row end),
        # double-buffered by row parity
        pyz = st.enter_context(nc.psum_tensor("pyz", [P, 2 * P], f32))

        dCst = st.enter_context(nc.semaphore("dCst"))
        sInit = st.enter_context(nc.semaphore("sInit"))
        # per-transfer DMA-completion sems (16 incs each)
        dC = [st.enter_context(nc.semaphore(f"dC{g}")) for g in range(4)]
        dIn = [st.enter_context(nc.semaphore(f"dIn{r}"))
               for r in range(1, ROWS)]
        dOut = [st.enter_context(nc.semaphore(f"dOut{i}")) for i in range(4)]
        sTp = st.enter_context(nc.semaphore("sTp"))  # +1 per transpose group
        sXt = st.enter_context(nc.semaphore("sXt"))  # +1 per px->xt copy group
        sMm = st.enter_context(nc.semaphore("sMm"))  # +1 per matmul
        sCp = st.enter_context(nc.semaphore("sCp"))  # +1 per ACT copy batch

        blk = st.enter_context(nc.Block())

        @blk.gpsimd
        def _(gp):
            # in-DMA on the gpsimd DGE ring: in bf16 mode these casts
            # f32 -> bf16 inline (only gpsimd can initiate casting DMAs).
            # Consts go first (they gate everything); row 0 in 4 chunks
            # so PE can start transposing early; rows 3+ wait for row 0
            # to land so its chunks aren't crowded out of the ring.
            gp.dma_start(out=ident_f[:], in_=tid[:]).then_inc(dCst, 16)
            gp.dma_start(out=tab_f[:], in_=tab[:]).then_inc(dCst, 16)
            for g in range(4):
                gp.dma_start(out=nat[0][:, g * 512:(g + 1) * 512],
                             in_=rb(xv[0][:, g * 512:(g + 1) * 512])
                             ).then_inc(dC[g], 16)
            for r in range(1, ROWS):
                if r == 3:
                    gp.wait_ge(dC[3], 16)
                gp.dma_start(out=nat[r][:],
                             in_=rb(xv[r])).then_inc(dIn[r - 1], 16)

        def slot(b):
            s = (b - 1) % 12
            return pyr[s // 4][:, (s % 4) * P:(s % 4 + 1) * P]

        def pyzr(r):
            return pyz[:, (r % 2) * P:(r % 2 + 1) * P]

        @blk.tensor
        def _(pe):
            pe.wait_ge(dCst, 32)
            pe.wait_ge(sInit, 1)
            for r in range(ROWS):
                # in-transposes: group g covers blocks 4g..4g+3
                for g in range(4):
                    gg = 4 * r + g
                    if r == 0:
                        pe.wait_ge(dC[g], 16)
                    elif g == 0:
                        pe.wait_ge(dIn[r - 1], 16)
                    if gg >= 3:
                        pe.wait_ge(sXt, gg - 2)  # px[gg%3] freed
                    dst = px[gg % 3]
                    for jj in range(4):
                        j = 4 * g + jj
                        ins = pe.transpose(
                            dst[:, jj * P:(jj + 1) * P],
                            nat[r][:, j * P:(j + 1) * P],
                            identr[:])
                    ins.then_inc(sTp, 1)
                # matmul pairs per stationary j: A(j) then B(j+1)/seam.
                # mm idx within row: A(b)=2b, B(b)=2b-1, seam=31.
                for j in range(NBLK):
                    pe.wait_ge(sXt, 4 * r + j // 4 + 1)
                    if j == 0:
                        # ring slots 0..2 freed by row r-1's copy batch 4
                        # (their last writers were blocks 13..15); pyz
                        # region r%2 freed by row r-2's copy batch 5.
                        if r >= 1:
                            pe.wait_ge(sCp, 5 * r - 1)
                        lhs = xt[r % 2][:, 0:P]
                        ins = pe.matmul(pyzr(r), lhs, tabr[:, 0:P],
                                        start=True, stop=False,
                                        skip_group_check=True)
                        ins.then_inc(sMm, 1)  # A(0)
                        ins = pe.matmul(slot(1), lhs, tabr[:, P:2 * P],
                                        start=True, stop=False,
                                        skip_group_check=True)
                        ins.then_inc(sMm, 1)  # B(1)
                    elif j < NBLK - 1:
                        # j==12: B(13)->slot 0, freed by THIS row's copy
                        # batch 1 (blocks 1..4 -> slots 0..3)
                        if j == 12:
                            pe.wait_ge(sCp, 5 * r + 1)
                        lhs = xt[r % 2][:, j * P:(j + 1) * P]
                        ins = pe.matmul(slot(j), lhs, tabr[:, 0:P],
                                        start=False, stop=True,
                                        skip_group_check=True)
                        ins.then_inc(sMm, 1)  # A(j)
                        ins = pe.matmul(slot(j + 1), lhs, tabr[:, P:2 * P],
                                        start=True, stop=False,
                                        skip_group_check=True)
                        ins.then_inc(sMm, 1)  # B(j+1)
                    else:
                        # j==15: A(15) from shifted block 15, then seam-B
                        ins = pe.matmul(slot(15), xt[r % 2][:, 1921:2049],
                                        tabr[:, 0:P],
                                        start=False, stop=True,
                                        skip_group_check=True)
                        ins.then_inc(sMm, 1)  # A(15)
                        ins = pe.matmul(pyzr(r), xt[r % 2][:, 1920:2048],
                                        tabr[:, P:2 * P],
                                        start=False, stop=True,
                                        skip_group_check=True)
                        ins.then_inc(sMm, 1)  # seam -> block 0

        @blk.vector
        def _(dve):
            dve.wait_ge(dCst, 32)
            if rmode or bmode:
                dve.tensor_copy(tabr[:], tab_f[:])
                dve.tensor_copy(identr[:], ident_f[:])
            # permanent seam guard columns
            gv = (lambda ap: ap.bitcast(f32)) if rmode else (lambda ap: ap)
            dve.memset(gv(xt[0][:, 1920:1921]), 0.0)
            ins = dve.memset(gv(xt[1][:, 1920:1921]), 0.0)
            ins.then_inc(sInit, 1)

            for r in range(ROWS):
                if r >= 2:
                    dve.wait_ge(sMm, 32 * (r - 1))  # xt[r%2] still read
                for g in range(4):
                    gg = 4 * r + g
                    dve.wait_ge(sTp, gg + 1)
                    # px holds f32 bits; out dtype f32r => this copy IS
                    # the f32->f32r rounding for the matmul stationary
                    pxs = xb(px[gg % 3][:])
                    if g == 3:
                        dve.tensor_copy(xt[r % 2][:, 1536:1920],
                                        pxs[:, 0:384])
                        ins = dve.tensor_copy(xt[r % 2][:, 1921:2049],
                                              pxs[:, 384:512])
                    else:
                        ins = dve.tensor_copy(
                            xt[r % 2][:, g * 512:(g + 1) * 512], pxs)
                    ins.then_inc(sXt, 1)

        @blk.scalar
        def _(act):
            for r in range(ROWS):
                # nato[r%4] free: out-DMA of row r-4 done
                if r >= 4:
                    act.wait_ge(dOut[r % 4], 32 * (r // 4))
                # copy batches: {1-4}{5-8}{9-12}{13-15}{0}
                batches = [
                    (32 * r + 9, pyr[0][:, 0:512], 1, 4),
                    (32 * r + 17, pyr[1][:, 0:512], 5, 4),
                    (32 * r + 25, pyr[2][:, 0:512], 9, 4),
                    (32 * r + 31, pyr[0][:, 0:384], 13, 3),
                    (32 * r + 32, pyzr(r), 0, 1),
                ]
                for (mmw, src, b0, nb) in batches:
                    act.wait_ge(sMm, mmw)
                    ins = act.copy(out=nato[r % 4][:, b0 * P:(b0 + nb) * P],
                                   in_=src)
                    ins.then_inc(sCp, 1)

        @blk.sync
        def _(sp):
            # out-DMA triggers on the (otherwise idle) SP HWDGE ring,
            # keeping the scalar engine free for PSUM->SBUF copies
            for r in range(ROWS):
                last = r == ROWS - 1
                if last:
                    # per-batch stores to shorten the tail
                    for k, (b0, nb) in enumerate([(1, 4), (5, 4), (9, 4),
                                                  (13, 3)]):
                        sp.wait_ge(sCp, 5 * r + k + 1)
                        sp.dma_start(out=yv[r][:, b0 * P:(b0 + nb) * P],
                                     in_=nato[r % 4][:, b0 * P:(b0 + nb) * P]
                                     ).then_inc(dOut[r % 4], 16)
                else:
                    sp.wait_ge(sCp, 5 * r + 4)
                    sp.dma_start(out=yv[r][:, P:M],
                                 in_=nato[r % 4][:, P:M]
                                 ).then_inc(dOut[r % 4], 16)
                sp.wait_ge(sCp, 5 * r + 5)
                sp.dma_start(out=yv[r][:, 0:P],
                             in_=nato[r % 4][:, 0:P]
                             ).then_inc(dOut[r % 4], 16)
            # drain: buffers 0..2 served rows {0,4},{1,5},{2,6} = 2x32;
            # buffer 3 rows {3,7} = 32 + 80
            sp.wait_ge(dOut[0], 64)
            sp.wait_ge(dOut[1], 64)
            sp.wait_ge(dOut[2], 64)
            sp.wait_ge(dOut[3], 112)

    return nc


def _get_nc():
    key = MM_MODE
    if key not in _cache:
        _cache[key] = _build(MM_MODE)
    return _cache[key]


def kernel(**inputs):
    from concourse.bass_utils import run_bass_kernel_spmd

    x = np.ascontiguousarray(np.asarray(inputs["x"], dtype=np.float32))
    assert x.shape == (BATCH, L), x.shape
    h = _taps(float(np.asarray(inputs["g_param"]).reshape(-1)[0]),
              float(np.asarray(inputs["R_param"]).reshape(-1)[0]),
              float(np.asarray(inputs["m_hp"]).reshape(-1)[0]),
              float(np.asarray(inputs["m_bp"]).reshape(-1)[0]),
              float(np.asarray(inputs["m_lp"]).reshape(-1)[0]))
    A, B = _toeplitz_mats(h)
    ident = np.eye(P, dtype=np.float32)
    common = {"tid": ident, "tab": np.concatenate([A, B], axis=1)}

    nc = _get_nc()
    core_ids = list(range(N_CORES))
    in_maps = [
        {"x": x[i * ROWS:(i + 1) * ROWS], **common}
        for i in range(N_CORES)
    ]
    kwargs = {}
    if TRACE:
        kwargs["tmpdir"] = os.environ.get("DSVF_TRACE_DIR") or None
    res = run_bass_kernel_spmd(nc, in_maps, core_ids, trace=TRACE, **kwargs)
    if TRACE:
        kernel.last_exec_time_ns = res.exec_time_ns
        kernel.last_results = res
    out = np.concatenate([res.results[i]["y"] for i in range(N_CORES)], axis=0)
    return out.astype(np.float32, copy=False)


kernel.last_exec_time_ns = None
